# revision 29
# baseline (speedup 1.0000x reference)
"""Trainium2 Bass kernel for a 3-layer TransformerConv GNN (nn_EncoderTransformerConv).

Strategy (8 NeuronCores, SPMD, identical instruction stream per core):
  - Nodes are partitioned across cores; each core's 6250 dst nodes are
    re-grouped (host bin-packing) into 49 blocks of 128 so that every
    (src-group, dst-block) edge group exactly fills its statically-sized
    tile budget -> the Q7 dma_gather scans a near-minimal number of index
    slots (the gather ucode at ~7.75 ns/slot is the kernel's critical
    resource).
  - Edges are grouped by 8 src-groups = (src quarter) x (src A/B region),
    where the A region of every core is the block range covered by the
    FIRST AllGather collective.  All A-region edge work is scheduled before
    any B-region work, so the second (layer-end) collective and the next
    layer's B-table builds overlap the A-region edge phase.
  - Per layer:
      Phase A  : every core computes the k|v projections for ALL nodes
                 (replicated dense bf16 matmuls) into 8 DRAM tables
                 (per quarter x region; <=8704 rows -> int16 indices); also
                 q (pre-scaled by 1/sqrt(C)) and root-skip s for OWN nodes.
      Phase B  : per chunk: dma_gather of kv[src] rows; one-hot S / ST
                 matrices generated ON-CHIP with DVE is_equal compares from
                 compact int8 dloc tensors (no one-hot DMA); q broadcast to
                 edges via ST matmul (4 tiles share one PSUM bank so the
                 q*k product is one DVE op per 4 tiles); alpha reduce; exp;
                 ex*v; one-hot S matmul reduces into PSUM per group,
                 accumulated across the 8 src-groups in an SBUF partial.
      Epilogue : normalize, mean heads, add skip, relu.
  - Pad slots carry dloc=-1 (their one-hot columns are all-zero, so they are
    dropped by the S matmul); chunk-trailing pads carry index -1, which the
    gather ucode strips.  SBUF gather buffers are memset once (layer 1) so
    never-written pad rows hold finite values.
"""
import os
import sys

sys.path.insert(0, "/opt/trn_rl_repo")

import ml_dtypes
import numpy as np

import concourse.bass as bass
import concourse.bacc as bacc
import concourse.mybir as mybir
import concourse.tile as tile
from concourse import bass_utils, library_config
from concourse.masks import make_identity

F32 = mybir.dt.float32
BF16 = mybir.dt.bfloat16
I16 = mybir.dt.int16
I8 = mybir.dt.int8
AF = mybir.ActivationFunctionType
OP = mybir.AluOpType
BNP = ml_dtypes.bfloat16

# ----- problem dims (hardcoded per spec) -----
SPEC = dict(N=50000, E=800000, D_IN=128, HID=64, H=2, M=8)
TILE_E = 128          # edges per tile
CHUNK_T = 24          # max tiles per edge-phase chunk (whole groups)
BLK = 128             # dst nodes per block
NG = 3                # src groups: A ranks 0-3 | A ranks 4-7 | B all ranks
SPLIT_B = 34          # blocks covered by first collective (A region)
PA_CHUNK = 7          # node tiles per phase-A chunk
QE_G = 4              # qe tiles sharing one PSUM bank


def _derive(cfg):
    d = dict(cfg)
    d["C"] = d["HID"]
    d["F"] = d["H"] * d["C"]            # 128 = q/k/v width
    d["WC"] = 3 * d["F"] + d["HID"]     # 448 packed k|v|q|s
    d["KV"] = 2 * d["F"]                # 256
    d["NPC_REAL"] = d["N"] // d["M"]
    d["NBLK"] = -(-d["NPC_REAL"] // BLK)
    d["NPC"] = d["NBLK"] * BLK
    d["NPAD"] = d["M"] * d["NPC"]
    d["RHSW"] = d["H"] + d["F"]         # 130 = denom cols + exv cols
    d["COLA"] = SPLIT_B * BLK           # A-region cols per core
    d["COLB"] = d["NPC"] - d["COLA"]
    d["TABA"] = (d["M"] // 2) * d["COLA"]   # rows per A table (rank quad)
    d["TABB"] = d["M"] * d["COLB"]          # rows of the single B table
    assert d["TABA"] < 2 ** 15 and d["TABB"] < 2 ** 15
    # original-id split of each core's nodes into A/B pools
    d["ASPLIT"] = d["NPC_REAL"] - (d["COLB"] - 15)
    return d


def _wrap_idx(a):
    """[M, n] int -> wrapped idx layout [M, 128, n//16]."""
    Mn, n = a.shape
    w = a.reshape(Mn, n // 16, 16).transpose(0, 2, 1)
    return np.ascontiguousarray(np.tile(w, (1, 8, 1))).astype(np.int16)


def _assign_pool(deg, caps, nblk):
    """Nodes (rows of deg [n, G]) -> nblk blocks of <=128 nodes with
    per-(block, g) loads <= caps [nblk, G].  Soft greedy, then swap-repair
    local search.  Returns blk or None."""
    n, G = deg.shape
    rng = np.random.default_rng(0)
    order = np.argsort(-deg.sum(1), kind="stable")
    L = np.zeros((nblk, G), np.int64)
    cnt = np.zeros(nblk, np.int64)
    blk = np.full(n, -1, np.int64)
    capf = caps.astype(np.float64)
    for i in order:
        need = deg[i]
        ok = cnt < BLK
        ratio = ((L + need) / capf).max(axis=1)
        ratio[~ok] = 1e18
        b = int(np.argmin(ratio))
        blk[i] = b
        L[b] += need
        cnt[b] += 1
    # swap-repair
    for _ in range(40000):
        viol = L - caps
        vb, vg = np.unravel_index(np.argmax(viol), viol.shape)
        if viol[vb, vg] <= 0:
            return blk
        in_b = np.nonzero(blk == vb)[0]
        # pick a heavy-in-g node of vb, swap with a light node elsewhere
        cand_i = in_b[np.argsort(-deg[in_b, vg])[:8]]
        best = None
        for i in cand_i:
            di = deg[i]
            out = np.nonzero(blk != vb)[0]
            if len(out) > 3000:
                out = rng.choice(out, 3000, replace=False)
            dj = deg[out]
            b2 = blk[out]
            # new loads after swap
            nb = L[vb] - di + dj                  # [k, G]
            n2 = L[b2] + di - dj
            v_new = (np.maximum(nb - caps[vb], 0).sum(1)
                     + np.maximum(n2 - caps[b2], 0).sum(1)
                     + np.maximum(L - caps, 0).sum()
                     - np.maximum(L[vb] - caps[vb], 0).sum()
                     - np.maximum(L[b2] - caps[b2], 0).sum(1))
            j = int(np.argmin(v_new))
            if best is None or v_new[j] < best[0]:
                best = (v_new[j], i, out[j])
        cur = np.maximum(L - caps, 0).sum()
        if best is None or best[0] >= cur:
            return None
        _, i, j = best
        bi, bj = blk[i], blk[j]
        L[bi] += deg[j] - deg[i]
        L[bj] += deg[i] - deg[j]
        blk[i], blk[j] = bj, bi
    return None


def _prep(x, edge_index, weights, d):
    M, NPC_REAL, NPC, NPAD, NBLK = (
        d["M"], d["NPC_REAL"], d["NPC"], d["NPAD"], d["NBLK"])
    N, D_IN, ASPLIT, COLA, COLB = (
        d["N"], d["D_IN"], d["ASPLIT"], d["COLA"], d["COLB"])

    src = np.asarray(edge_index[0]).astype(np.int64)
    dst = np.asarray(edge_index[1]).astype(np.int64)
    core = dst // NPC_REAL
    scc = src // NPC_REAL
    sreg = ((src - scc * NPC_REAL) >= ASPLIT).astype(np.int64)
    sg = np.where(sreg == 1, 2, scc // (M // 2))   # A0 | A1 | B

    deg = np.zeros((N, NG), np.int64)
    np.add.at(deg, (dst, sg), 1)

    # per-(core, group, dst-pool) edge counts -> per-pool cap budgets
    dpool = ((dst - core * NPC_REAL) >= ASPLIT).astype(np.int64)
    Emgp = np.zeros((M, NG, 2), np.int64)
    np.add.at(Emgp, (core, sg, dpool), 1)
    needp = Emgp.max(axis=0)                  # [NG, 2]

    # per-(group, block) tile budgets: minimal per-pool total, rotated bumps
    T8 = np.zeros((NG, NBLK), np.int64)
    for g in range(NG):
        for p, (b0, b1) in enumerate(((0, SPLIT_B), (SPLIT_B, NBLK))):
            nb_blk = b1 - b0
            nd = int(needp[g, p]) + 64
            base = max(1, nd // (nb_blk * BLK))
            T8[g, b0:b1] = base
            extra = nd - base * nb_blk * BLK
            nb = max(0, -(-extra // BLK))
            for j in range(nb):
                T8[g, b0 + (g * 5 + j) % nb_blk] += 1

    for _attempt in range(8):
        capsA = (T8 * BLK).T[:SPLIT_B]        # [34, NG]
        capsB = (T8 * BLK).T[SPLIT_B:]
        blks = []
        ok = True
        for m in range(M):
            nd = np.arange(m * NPC_REAL, (m + 1) * NPC_REAL)
            bA = _assign_pool(deg[nd[:ASPLIT]], capsA, SPLIT_B)
            bB = _assign_pool(deg[nd[ASPLIT:]], capsB, NBLK - SPLIT_B)
            if bA is None or bB is None:
                ok = False
                break
            blks.append(np.concatenate([bA, bB + SPLIT_B]))
        if ok:
            break
        T8[:, (7 * _attempt) % NBLK] += 1     # grow every group, retry
    assert ok, "bin packing failed"

    # published node order
    pid = np.zeros(N, np.int64)
    slot_of = np.zeros(N, np.int64)
    for m in range(M):
        nd = np.arange(m * NPC_REAL, (m + 1) * NPC_REAL)
        b = blks[m]
        order = np.lexsort((nd, b))
        sl = np.zeros(NPC_REAL, np.int64)
        bb = b[order]
        newb = np.ones(NPC_REAL, bool)
        newb[1:] = bb[1:] != bb[:-1]
        starts = np.nonzero(newb)[0]
        runid = np.cumsum(newb) - 1
        sl[order] = np.arange(NPC_REAL) - starts[runid]
        assert sl.max() < BLK
        pid[nd] = m * NPC + b * BLK + sl
        slot_of[nd] = sl

    # src table row: A tables hold 4 rank slabs; B table holds all 8
    ppos = pid[src]
    prk = ppos // NPC
    pb = (ppos % NPC) // BLK
    psl = ppos % BLK
    isA = pb < SPLIT_B
    rowA = (prk % (M // 2)) * COLA + pb * BLK + psl
    rowB = prk * COLB + (pb - SPLIT_B) * BLK + psl
    srow = np.where(isA, rowA, rowB)
    assert (sreg == (~isA).astype(np.int64)).all()

    # group/tile/chunk structure
    groups = []            # (g, b, t0, T)
    tmeta = []             # per tile: (seg, b, st, sp)  seg = g
    t = 0
    for g in range(NG):
        for b in range(NBLK):
            tq = int(T8[g, b])
            groups.append((g, b, t, tq))
            for i in range(tq):
                tmeta.append((g, b, i == 0, i == tq - 1))
            t += tq
    TT = t
    chunks = []            # (t0, nt, g)
    gi = 0
    while gi < len(groups):
        g, b, t0, tq = groups[gi]
        nt = tq
        gj = gi + 1
        while (gj < len(groups) and groups[gj][0] == g
               and nt + groups[gj][3] <= CHUNK_T):
            nt += groups[gj][3]
            gj += 1
        chunks.append((t0, nt, g))
        gi = gj
    group_t0 = {(g, b): (t0, tq) for (g, b, t0, tq) in groups}

    # place edges
    bof = np.empty(len(dst), np.int64)
    for m in range(M):
        selm = core == m
        bof[selm] = blks[m][dst[selm] - m * NPC_REAL]
    key = (core * NG + sg) * NBLK + bof
    order = np.argsort(key, kind="stable")
    sk = key[order]
    new_run = np.ones(len(sk), bool)
    new_run[1:] = sk[1:] != sk[:-1]
    starts = np.nonzero(new_run)[0]
    runid = np.cumsum(new_run) - 1
    rank = np.arange(len(sk)) - starts[runid]
    t0g = np.array([group_t0[(g, b)][0] for g in range(NG)
                    for b in range(NBLK)]).reshape(NG, NBLK)
    pos = t0g[sg[order], bof[order]] * TILE_E + rank
    assert (rank < (T8 * BLK)[sg[order], bof[order]]).all(), "group overflow"

    kv_idx = np.zeros((M, TT * TILE_E), np.int64)
    dloc = np.full((M, TT * TILE_E), -1, np.int64)
    kv_idx[core[order], pos] = srow[order]
    dloc[core[order], pos] = slot_of[dst[order]]
    assert kv_idx.max() < 2 ** 15

    # trailing pads of each chunk -> idx -1 (stripped by the gather ucode)
    cnts = np.zeros((M, NG, NBLK), np.int64)
    np.add.at(cnts, (core, sg, bof), 1)
    pad_tiles = []
    for (t0, nt, g) in chunks:
        padt = 0
        for (gg, b, g0, tq) in groups:
            if gg == g and g0 + tq == t0 + nt:
                for m in range(M):
                    c = int(cnts[m, g, b])
                    lo = g0 * TILE_E + c
                    hi = (g0 + tq) * TILE_E
                    if lo < hi:
                        kv_idx[m, lo:hi] = -1
                    padt = max(padt, -(-(hi - lo) // TILE_E))
                break
        pad_tiles.append(padt)

    kv_w = _wrap_idx(kv_idx)

    # one-hot S / ST in fp8 (exact 0/1); S: [e, t*128+d], ST: [d, t*128+e]
    FP8NP = ml_dtypes.float8_e4m3fn
    S_in = np.zeros((M, 128, TT * TILE_E), FP8NP)
    ST_in = np.zeros((M, 128, TT * TILE_E), FP8NP)
    dl = dloc.reshape(M, TT, TILE_E)
    mm, tt_, pp_ = np.nonzero(dl >= 0)
    dv = dl[mm, tt_, pp_]
    S_in[mm, pp_, tt_ * TILE_E + dv] = 1.0
    ST_in[mm, dv, tt_ * TILE_E + pp_] = 1.0

    xT_pad = np.zeros((D_IN, NPAD), np.float32)
    xT_pad[:, pid] = np.asarray(x).T
    xT = xT_pad.astype(BNP)

    wt = {}
    for L in (1, 2, 3):
        W_all, b_all = weights[L]
        if L == 1:
            wt["W1"] = W_all.astype(BNP)
            wt["brep1"] = np.ascontiguousarray(
                np.tile(b_all[None, :], (128, 1)).astype(np.float32))
        else:
            wt[f"W{L}"] = np.concatenate(
                [W_all, b_all[None, :]], 0).astype(BNP)

    in_maps = []
    for m in range(M):
        im = dict(
            xT=np.ascontiguousarray(xT),
            kvidx=np.ascontiguousarray(kv_w[m]),
            S_in=np.ascontiguousarray(S_in[m]),
            ST_in=np.ascontiguousarray(ST_in[m]),
            **wt,
        )
        in_maps.append(im)

    meta = dict(TT=TT, tmeta=tmeta, chunks=chunks, T=T8, pid=pid,
                pad_tiles=pad_tiles)
    return in_maps, meta


def build_module(d, meta):
    TT, chunks, tmeta = meta["TT"], meta["chunks"], meta["tmeta"]
    pad_tiles = meta["pad_tiles"]
    M, NPC, NPAD, NBLK = d["M"], d["NPC"], d["NPAD"], d["NBLK"]
    D_IN, F, KV, WC, HID, H, C, RHSW = (
        d["D_IN"], d["F"], d["KV"], d["WC"], d["HID"], d["H"], d["C"],
        d["RHSW"])
    COLA, COLB, TABA, TABB = d["COLA"], d["COLB"], d["TABA"], d["TABB"]

    nc = bacc.Bacc("TRN2", target_bir_lowering=False, debug=False,
                   num_devices=M)
    inp = {}
    FP8 = mybir.dt.float8e4
    for name, shape, dt in [
        ("xT", [D_IN, NPAD], BF16), ("xoT", [D_IN, NPC], BF16),
        ("W1", [D_IN, WC], BF16), ("brep1", [128, WC], F32),
        ("W2", [HID + 1, WC], BF16), ("W3", [HID + 1, WC], BF16),
        ("kvidx", [128, TT * 8], I16),
        ("S_in", [128, TT * TILE_E], FP8), ("ST_in", [128, TT * TILE_E], FP8),
    ]:
        inp[name] = nc.dram_tensor(name, shape, dt, kind="ExternalInput")
    h_out = nc.dram_tensor("h_out", [NPC, HID], F32, kind="ExternalOutput")

    with tile.TileContext(nc) as tc:
        with tc.tile_pool(name="dram", bufs=1, space="DRAM") as dram, \
             tc.tile_pool(name="res", bufs=1) as res:
            tabs = [dram.tile([TABA, KV], BF16, name="tabA0"),
                    dram.tile([TABA, KV], BF16, name="tabA1"),
                    dram.tile([TABB, KV], BF16, name="tabB")]
            coll_inA = dram.tile([HID + 1, COLA], BF16)
            coll_outA = dram.tile([M * (HID + 1), COLA], BF16)
            coll_inB = dram.tile([HID + 1, COLB], BF16)
            coll_outB = dram.tile([M * (HID + 1), COLB], BF16)

            nc.gpsimd.load_library(library_config.mlp)

            W1_sb = res.tile([D_IN, WC], BF16)
            brep1_sb = res.tile([128, WC], F32)
            W2_sb = res.tile([HID + 1, WC], BF16)
            W3_sb = res.tile([HID + 1, WC], BF16)
            kvidx_sb = res.tile([128, TT * 8], I16)
            q_sb = res.tile([128, NBLK * F], BF16)
            s_sb = res.tile([128, NBLK * HID], F32)
            hTown = res.tile([HID + 1, NPC], BF16)
            partA = res.tile([128, NBLK * RHSW], F32)
            ident = res.tile([128, 128], F32)
            eps2 = res.tile([128, H], F32)

            for sb, t in ((W1_sb, "W1"), (brep1_sb, "brep1"), (W2_sb, "W2"),
                          (W3_sb, "W3"), (kvidx_sb, "kvidx")):
                nc.sync.dma_start(sb[:], inp[t].ap())
            make_identity(nc, ident[:])
            nc.vector.memset(hTown[HID:HID + 1, :], 1.0)
            nc.vector.memset(eps2[:], H * 1e-16)

            with tc.tile_pool(name="pa", bufs=4) as pa, \
                 tc.tile_pool(name="pap", bufs=2, space="PSUM") as pap, \
                 tc.tile_pool(name="pb", bufs=3) as pb, \
                 tc.tile_pool(name="pb1", bufs=2) as pb1, \
                 tc.tile_pool(name="pbp", bufs=2, space="PSUM") as pbp, \
                 tc.tile_pool(name="qep", bufs=2, space="PSUM") as qep, \
                 tc.tile_pool(name="epp", bufs=1, space="PSUM") as epp, \
                 tc.tile_pool(name="ep", bufs=2) as ep:
              for layer in (1, 2, 3):
                W_sb = {1: W1_sb, 2: W2_sb, 3: W3_sb}[layer]

                # ---------- Phase A ----------
                if True:
                    def emit_kv(tab, row0, ntl, la):
                        kvst = pa.tile([128, PA_CHUNK * KV], BF16, tag="kvst")
                        for t in range(ntl):
                            ps = pap.tile([128, KV], F32, tag="pskv",
                                          name="pskv")
                            dstp = kvst[:, t * KV:(t + 1) * KV]
                            nc.tensor.matmul(ps[:],
                                             la[:, t * 128:(t + 1) * 128],
                                             W_sb[:, 0:KV],
                                             start=True, stop=True)
                            nc.scalar.copy(dstp, ps[:])
                        if layer == 1:
                            nc.vector.tensor_tensor(
                                kvst[:, 0:ntl * KV].rearrange(
                                    "p (t e) -> p t e", e=KV),
                                kvst[:, 0:ntl * KV].rearrange(
                                    "p (t e) -> p t e", e=KV),
                                brep1_sb[:, 0:KV].rearrange(
                                    "p (t e) -> p t e", t=1).to_broadcast(
                                        [128, ntl, KV]),
                                op=OP.add)
                        nc.sync.dma_start(
                            tab[row0:row0 + ntl * 128, :].rearrange(
                                "(t p) e -> p t e", p=128),
                            kvst[:, 0:ntl * KV].rearrange(
                                "p (t e) -> p t e", e=KV))

                    def emit_region(reg):
                        colr, regt = ((COLA, COLA // 128) if reg == 0
                                      else (COLB, COLB // 128))
                        for rk in range(M):
                            ch = 0
                            while ch < regt:
                                ntl = min(PA_CHUNK, regt - ch)
                                if layer == 1:
                                    la = pa.tile([D_IN, PA_CHUNK * 128],
                                                 BF16, tag="la")
                                    c0 = rk * NPC + reg * COLA + ch * 128
                                    nc.sync.dma_start(
                                        la[:, 0:ntl * 128],
                                        inp["xT"].ap()[:, c0:c0 + ntl * 128])
                                else:
                                    la = pa.tile([HID + 1, PA_CHUNK * 128],
                                                 BF16, tag="la")
                                    reg_t = (coll_outA if reg == 0
                                             else coll_outB)
                                    nc.sync.dma_start(
                                        la[:, 0:ntl * 128],
                                        reg_t[rk * (HID + 1):
                                              (rk + 1) * (HID + 1),
                                              ch * 128:(ch + ntl) * 128])
                                if reg == 0:
                                    tab = tabs[rk // (M // 2)]
                                    row0 = (rk % (M // 2)) * colr + ch * 128
                                else:
                                    tab = tabs[2]
                                    row0 = rk * colr + ch * 128
                                emit_kv(tab, row0, ntl, la)
                                ch += ntl
                            if reg == 0 and rk == M // 2 - 1:
                                emit_qs()

                    def emit_qs():
                        for ch in range(0, NBLK, PA_CHUNK):
                            ntl = min(PA_CHUNK, NBLK - ch)
                            if layer == 1:
                                la = pa.tile([D_IN, PA_CHUNK * 128], BF16,
                                             tag="la")
                                # own cols inside xT (device id 0 layout is
                                # identical across cores; per-core data maps
                                # provide the right xT, and own cols are at
                                # device-rank offset) -- use replica-specific
                                # xoT slice of xT instead:
                                c0 = ch * 128
                                nc.sync.dma_start(
                                    la[:, 0:ntl * 128],
                                    inp["xoT"].ap()[:, c0:c0 + ntl * 128])
                            for t in range(ntl):
                                gt = ch + t
                                ps = pap.tile([128, KV], F32,
                                              tag="pskv", name="psqs")
                                if layer == 1:
                                    nc.tensor.matmul(
                                        ps[:, 0:F + HID],
                                        la[:, t * 128:(t + 1) * 128],
                                        W1_sb[:, KV:WC], start=True, stop=True)
                                else:
                                    nc.tensor.matmul(
                                        ps[:, 0:F + HID],
                                        hTown[:, gt * 128:(gt + 1) * 128],
                                        W_sb[:, KV:WC], start=True, stop=True)
                                nc.vector.tensor_copy(
                                    q_sb[:, gt * F:(gt + 1) * F],
                                    ps[:, 0:F])
                                nc.scalar.copy(
                                    s_sb[:, gt * HID:(gt + 1) * HID],
                                    ps[:, F:F + HID])
                            if layer == 1:
                                nc.vector.tensor_tensor(
                                    q_sb[:, ch * F:(ch + ntl) * F].rearrange(
                                        "p (t e) -> p t e", e=F),
                                    q_sb[:, ch * F:(ch + ntl) * F].rearrange(
                                        "p (t e) -> p t e", e=F),
                                    brep1_sb[:, KV:KV + F].rearrange(
                                        "p (t e) -> p t e", t=1).to_broadcast(
                                            [128, ntl, F]),
                                    op=OP.add)
                                nc.vector.tensor_tensor(
                                    s_sb[:, ch * HID:(ch + ntl) * HID
                                         ].rearrange("p (t e) -> p t e", e=HID),
                                    s_sb[:, ch * HID:(ch + ntl) * HID
                                         ].rearrange("p (t e) -> p t e", e=HID),
                                    brep1_sb[:, KV + F:WC].rearrange(
                                        "p (t e) -> p t e", t=1).to_broadcast(
                                            [128, ntl, HID]),
                                    op=OP.add)

                    emit_region(0)
                    emit_region(1)

                # ---------- Phase B ----------
                if True:
                    psum_g = None
                    for cj, (t0, nt, g) in enumerate(chunks):
                        n = nt * TILE_E
                        tab = tabs[g]
                        kvg = pb.tile([128, CHUNK_T, KV], BF16, tag="kvg")
                        rhs = pb.tile([128, CHUNK_T, RHSW], BF16, tag="rhs")
                        Sg = pb1.tile([128, CHUNK_T, 128], FP8, tag="Sg")
                        STg = pb1.tile([128, CHUNK_T, 128], FP8, tag="STg")
                        prod = pb1.tile([128, CHUNK_T * F], BF16, tag="prod")
                        alph = pb1.tile([128, CHUNK_T * H], F32, tag="alph")

                        if layer == 1 and cj < 3:
                            nc.vector.memset(
                                kvg[:].rearrange("p a b -> p (a b)"), 0.0)
                        nc.gpsimd.dma_gather(
                            out_ap=kvg[:, 0:nt, :], in_ap=tab[:],
                            idxs_ap=kvidx_sb[:, t0 * 8:t0 * 8 + nt * 8],
                            num_idxs=n, num_idxs_reg=n, elem_size=KV,
                            single_packet=False)
                        nc.sync.dma_start(
                            Sg[:, 0:nt, :].rearrange("p a b -> p (a b)"),
                            inp["S_in"].ap()[:, t0 * TILE_E:t0 * TILE_E + n])
                        nc.sync.dma_start(
                            STg[:, 0:nt, :].rearrange("p a b -> p (a b)"),
                            inp["ST_in"].ap()[:, t0 * TILE_E:t0 * TILE_E + n])

                        i = 0
                        while i < nt:
                            ng = min(QE_G, nt - i)
                            qeg = qep.tile([128, QE_G * F], F32, name="qeg",
                                           tag="qeg")
                            for j in range(ng):
                                b = tmeta[t0 + i + j][1]
                                nc.tensor.matmul(
                                    qeg[:, j * F:(j + 1) * F],
                                    STg[:, i + j, :],
                                    q_sb[:, b * F:(b + 1) * F],
                                    start=True, stop=True)
                            nc.vector.tensor_tensor(
                                out=prod[:, i * F:(i + ng) * F].rearrange(
                                    "p (t f) -> p t f", f=F),
                                in0=qeg[:, 0:ng * F].rearrange(
                                    "p (t f) -> p t f", f=F),
                                in1=kvg[:, i:i + ng, 0:F],
                                op=OP.mult)
                            i += ng
                        nc.vector.reduce_sum(
                            alph[:, 0:nt * H].rearrange("p (t h) -> p t h",
                                                        h=H),
                            prod[:, 0:nt * F].rearrange(
                                "p (t h c) -> p t h c", h=H, c=C),
                            axis=mybir.AxisListType.X)
                        nc.scalar.activation(
                            rhs[:, 0:nt, 0:H],
                            alph[:, 0:nt * H].rearrange("p (t h) -> p t h",
                                                        h=H),
                            AF.Exp)
                        nc.vector.tensor_tensor(
                            out=rhs[:, 0:nt, H:RHSW].rearrange(
                                "p t (h c) -> p t h c", c=C),
                            in0=kvg[:, 0:nt, F:KV].rearrange(
                                "p t (h c) -> p t h c", c=C),
                            in1=rhs[:, 0:nt, 0:H].to_broadcast(
                                [128, nt, H, C]),
                            op=OP.mult)

                        for i in range(nt):
                            gg, b, st, sp = tmeta[t0 + i]
                            if st:
                                psum_g = pbp.tile([128, RHSW], F32,
                                                  name="pblk", tag="pblk")
                            nc.tensor.matmul(
                                psum_g[:], Sg[:, i, :], rhs[:, i, :],
                                start=st, stop=sp)
                            if not sp:
                                continue
                            pa_sl = partA[:, b * RHSW:(b + 1) * RHSW]
                            if gg == 0:
                                nc.scalar.copy(pa_sl, psum_g[:])
                                continue
                            if gg < NG - 1:
                                nc.vector.tensor_tensor(pa_sl, psum_g[:],
                                                        pa_sl, op=OP.add)
                                continue
                            # ---- epilogue for block b ----
                            tot = ep.tile([128, RHSW], F32, tag="tot")
                            nc.vector.tensor_tensor(tot[:], psum_g[:], pa_sl,
                                                    op=OP.add)
                            rec = ep.tile([128, H], F32, tag="rec")
                            nc.vector.scalar_tensor_tensor(
                                out=rec[:], in0=tot[:, 0:H], scalar=float(H),
                                in1=eps2[:], op0=OP.mult, op1=OP.add)
                            nc.vector.reciprocal(rec[:], rec[:])
                            m0 = ep.tile([128, C], F32, tag="m0")
                            nc.vector.scalar_tensor_tensor(
                                out=m0[:], in0=tot[:, H:H + C],
                                scalar=rec[:, 0:1],
                                in1=s_sb[:, b * HID:(b + 1) * HID],
                                op0=OP.mult, op1=OP.add)
                            hp2 = ep.tile([128, HID], F32, tag="hp2")
                            nc.vector.scalar_tensor_tensor(
                                out=hp2[:], in0=tot[:, H + C:H + 2 * C],
                                scalar=rec[:, 1:2], in1=m0[:],
                                op0=OP.mult, op1=OP.add)
                            hblk = ep.tile([128, HID], F32, tag="hblk")
                            nc.scalar.activation(hblk[:], hp2[:], AF.Relu)
                            if layer < 3:
                                pst = epp.tile([HID, 128], F32)
                                nc.tensor.transpose(pst[:], hblk[:], ident[:])
                                nc.vector.tensor_copy(
                                    hTown[0:HID, b * 128:(b + 1) * 128],
                                    pst[:])
                                if b == SPLIT_B - 1:
                                    nc.sync.dma_start(coll_inA[:, :],
                                                      hTown[:, 0:COLA])
                                    nc.gpsimd.collective_compute(
                                        "AllGather", OP.bypass,
                                        ins=[coll_inA.opt()],
                                        outs=[coll_outA.opt()],
                                        replica_groups=[list(range(M))])
                            else:
                                nc.sync.dma_start(
                                    h_out.ap()[b * 128:(b + 1) * 128, :],
                                    hblk[:])

                if layer < 3:
                    nc.sync.dma_start(coll_inB[:, :], hTown[:, COLA:])
                    nc.gpsimd.collective_compute(
                        "AllGather", OP.bypass,
                        ins=[coll_inB.opt()], outs=[coll_outB.opt()],
                        replica_groups=[list(range(M))])
    nc.compile()
    return nc


# ---------------- public entry ----------------
_CACHE = {}


def _weights_from_inputs(inputs, d):
    # packed column order: k | v | q | s ; q pre-scaled by 1/sqrt(C)
    sc = 1.0 / np.sqrt(d["C"])
    wt = {}
    for L in (1, 2, 3):
        Ws, bs = [], []
        for nm in ("k", "v", "q", "s"):
            W = np.asarray(inputs[f"W{L}{nm}"], np.float32)
            b = np.asarray(inputs[f"b{L}{nm}"], np.float32)
            if nm == "q":
                W = W * sc
                b = b * sc
            Ws.append(W)
            bs.append(b)
        wt[L] = (np.concatenate(Ws, axis=1), np.concatenate(bs))
    return wt


def _install_ntff_shim():
    import types
    if "antenv.axon_hooks" in sys.modules:
        return
    try:
        from trn_agent_boot.trn_boot import _ntff_profile_via_ctypes
        hook = _ntff_profile_via_ctypes("/opt/axon/libaxon_pjrt.so")
    except Exception:
        hook = None
    mod = types.ModuleType("antenv.axon_hooks")
    mod.get_axon_ntff_profile_hook = lambda: hook
    mod.set_axon_ntff_profile_hook = lambda h: None
    sys.modules["antenv.axon_hooks"] = mod
    try:
        import antenv
        antenv.axon_hooks = mod
    except Exception:
        pass


def run(inputs, cfg=SPEC, trace=False):
    d = _derive(cfg)
    wt = _weights_from_inputs(inputs, d)
    in_maps, meta = _prep(inputs["x"], inputs["edge_index"], wt, d)
    for m in range(d["M"]):
        in_maps[m]["xoT"] = np.ascontiguousarray(
            in_maps[m]["xT"][:, m * d["NPC"]:(m + 1) * d["NPC"]])
    key = (tuple(sorted(cfg.items())), meta["TT"],
           tuple(meta["T"].flatten().tolist()))
    if key not in _CACHE:
        _CACHE[key] = build_module(d, meta)
    nc = _CACHE[key]
    if trace:
        _install_ntff_shim()
    res = bass_utils.run_bass_kernel_spmd(
        nc, in_maps, core_ids=list(range(d["M"])), trace=trace)
    pid = meta["pid"]
    N, NPC = d["N"], d["NPC"]
    full = np.empty((N, d["HID"]), np.float32)
    for m in range(d["M"]):
        sel = np.arange(m * d["NPC_REAL"], (m + 1) * d["NPC_REAL"])
        full[sel] = res.results[m]["h_out"][pid[sel] - m * NPC]
    return full, res


def kernel(**inputs) -> np.ndarray:
    trace = bool(os.environ.get("KERNEL_TRACE"))
    full, res = run(inputs, SPEC, trace=trace)
    if trace and res.exec_time_ns is not None:
        print(f"HW exec time: {res.exec_time_ns} ns")
    return full


# revision 30
# speedup vs baseline: 1.0692x; 1.0692x over previous
"""Trainium2 Bass kernel for a 3-layer TransformerConv GNN (nn_EncoderTransformerConv).

Strategy (8 NeuronCores, SPMD, identical instruction stream per core):
  - Nodes are partitioned across cores; each core's 6250 dst nodes are
    re-grouped (host bin-packing) into 49 blocks of 128 so that every
    (src-group, dst-block) edge group exactly fills its statically-sized
    tile budget -> the Q7 dma_gather scans a near-minimal number of index
    slots (the gather ucode at ~7.75 ns/slot is the kernel's critical
    resource).
  - Edges are grouped by 8 src-groups = (src quarter) x (src A/B region),
    where the A region of every core is the block range covered by the
    FIRST AllGather collective.  All A-region edge work is scheduled before
    any B-region work, so the second (layer-end) collective and the next
    layer's B-table builds overlap the A-region edge phase.
  - Per layer:
      Phase A  : every core computes the k|v projections for ALL nodes
                 (replicated dense bf16 matmuls) into 8 DRAM tables
                 (per quarter x region; <=8704 rows -> int16 indices); also
                 q (pre-scaled by 1/sqrt(C)) and root-skip s for OWN nodes.
      Phase B  : per chunk: dma_gather of kv[src] rows; one-hot S / ST
                 matrices generated ON-CHIP with DVE is_equal compares from
                 compact int8 dloc tensors (no one-hot DMA); q broadcast to
                 edges via ST matmul (4 tiles share one PSUM bank so the
                 q*k product is one DVE op per 4 tiles); alpha reduce; exp;
                 ex*v; one-hot S matmul reduces into PSUM per group,
                 accumulated across the 8 src-groups in an SBUF partial.
      Epilogue : normalize, mean heads, add skip, relu.
  - Pad slots carry dloc=-1 (their one-hot columns are all-zero, so they are
    dropped by the S matmul); chunk-trailing pads carry index -1, which the
    gather ucode strips.  SBUF gather buffers are memset once (layer 1) so
    never-written pad rows hold finite values.
"""
import os
import sys

sys.path.insert(0, "/opt/trn_rl_repo")

import ml_dtypes
import numpy as np

import concourse.bass as bass
import concourse.bacc as bacc
import concourse.mybir as mybir
import concourse.tile as tile
from concourse import bass_utils, library_config
from concourse.masks import make_identity

F32 = mybir.dt.float32
BF16 = mybir.dt.bfloat16
I16 = mybir.dt.int16
I8 = mybir.dt.int8
AF = mybir.ActivationFunctionType
OP = mybir.AluOpType
BNP = ml_dtypes.bfloat16

# ----- problem dims (hardcoded per spec) -----
SPEC = dict(N=50000, E=800000, D_IN=128, HID=64, H=2, M=8)
TILE_E = 128          # edges per tile
CHUNK_T = 24          # max tiles per edge-phase chunk (whole groups)
BLK = 128             # dst nodes per block
NG = 3                # src groups: A ranks 0-3 | A ranks 4-7 | B all ranks
SPLIT_B = 26          # blocks covered by first collective (A region)
PA_CHUNK = 7          # node tiles per phase-A chunk
QE_G = 4              # qe tiles sharing one PSUM bank


def _derive(cfg):
    d = dict(cfg)
    d["C"] = d["HID"]
    d["F"] = d["H"] * d["C"]            # 128 = q/k/v width
    d["WC"] = 3 * d["F"] + d["HID"]     # 448 packed k|v|q|s
    d["KV"] = 2 * d["F"]                # 256
    d["NPC_REAL"] = d["N"] // d["M"]
    d["NBLK"] = -(-d["NPC_REAL"] // BLK)
    d["NPC"] = d["NBLK"] * BLK
    d["NPAD"] = d["M"] * d["NPC"]
    d["RHSW"] = d["H"] + d["F"]         # 130 = denom cols + exv cols
    d["COLA"] = SPLIT_B * BLK           # A-region cols per core
    d["COLB"] = d["NPC"] - d["COLA"]
    d["TABA"] = (d["M"] // 2) * d["COLA"]   # rows per A table (rank quad)
    d["TABB"] = d["M"] * d["COLB"]          # rows of the single B table
    assert d["TABA"] < 2 ** 15 and d["TABB"] < 2 ** 15
    # original-id split of each core's nodes into A/B pools
    d["ASPLIT"] = d["NPC_REAL"] - (d["COLB"] - 15)
    return d


def _wrap_idx(a):
    """[M, n] int -> wrapped idx layout [M, 128, n//16]."""
    Mn, n = a.shape
    w = a.reshape(Mn, n // 16, 16).transpose(0, 2, 1)
    return np.ascontiguousarray(np.tile(w, (1, 8, 1))).astype(np.int16)


def _assign_pool(deg, caps, nblk):
    """Nodes (rows of deg [n, G]) -> nblk blocks of <=128 nodes with
    per-(block, g) loads <= caps [nblk, G].  Soft greedy, then swap-repair
    local search.  Returns blk or None."""
    n, G = deg.shape
    rng = np.random.default_rng(0)
    order = np.argsort(-deg.sum(1), kind="stable")
    L = np.zeros((nblk, G), np.int64)
    cnt = np.zeros(nblk, np.int64)
    blk = np.full(n, -1, np.int64)
    capf = caps.astype(np.float64)
    for i in order:
        need = deg[i]
        ok = cnt < BLK
        ratio = ((L + need) / capf).max(axis=1)
        ratio[~ok] = 1e18
        b = int(np.argmin(ratio))
        blk[i] = b
        L[b] += need
        cnt[b] += 1
    # swap-repair
    for _ in range(40000):
        viol = L - caps
        vb, vg = np.unravel_index(np.argmax(viol), viol.shape)
        if viol[vb, vg] <= 0:
            return blk
        in_b = np.nonzero(blk == vb)[0]
        # pick a heavy-in-g node of vb, swap with a light node elsewhere
        cand_i = in_b[np.argsort(-deg[in_b, vg])[:8]]
        best = None
        for i in cand_i:
            di = deg[i]
            out = np.nonzero(blk != vb)[0]
            if len(out) > 3000:
                out = rng.choice(out, 3000, replace=False)
            dj = deg[out]
            b2 = blk[out]
            # new loads after swap
            nb = L[vb] - di + dj                  # [k, G]
            n2 = L[b2] + di - dj
            v_new = (np.maximum(nb - caps[vb], 0).sum(1)
                     + np.maximum(n2 - caps[b2], 0).sum(1)
                     + np.maximum(L - caps, 0).sum()
                     - np.maximum(L[vb] - caps[vb], 0).sum()
                     - np.maximum(L[b2] - caps[b2], 0).sum(1))
            j = int(np.argmin(v_new))
            if best is None or v_new[j] < best[0]:
                best = (v_new[j], i, out[j])
        cur = np.maximum(L - caps, 0).sum()
        if best is None or best[0] >= cur:
            return None
        _, i, j = best
        bi, bj = blk[i], blk[j]
        L[bi] += deg[j] - deg[i]
        L[bj] += deg[i] - deg[j]
        blk[i], blk[j] = bj, bi
    return None


def _prep(x, edge_index, weights, d):
    M, NPC_REAL, NPC, NPAD, NBLK = (
        d["M"], d["NPC_REAL"], d["NPC"], d["NPAD"], d["NBLK"])
    N, D_IN, ASPLIT, COLA, COLB = (
        d["N"], d["D_IN"], d["ASPLIT"], d["COLA"], d["COLB"])

    src = np.asarray(edge_index[0]).astype(np.int64)
    dst = np.asarray(edge_index[1]).astype(np.int64)
    core = dst // NPC_REAL
    scc = src // NPC_REAL
    sreg = ((src - scc * NPC_REAL) >= ASPLIT).astype(np.int64)
    sg = np.where(sreg == 1, 2, scc // (M // 2))   # A0 | A1 | B

    deg = np.zeros((N, NG), np.int64)
    np.add.at(deg, (dst, sg), 1)

    # per-(core, group, dst-pool) edge counts -> per-pool cap budgets
    dpool = ((dst - core * NPC_REAL) >= ASPLIT).astype(np.int64)
    Emgp = np.zeros((M, NG, 2), np.int64)
    np.add.at(Emgp, (core, sg, dpool), 1)
    needp = Emgp.max(axis=0)                  # [NG, 2]

    # per-(group, block) tile budgets: minimal per-pool total, rotated bumps
    T8 = np.zeros((NG, NBLK), np.int64)
    for g in range(NG):
        for p, (b0, b1) in enumerate(((0, SPLIT_B), (SPLIT_B, NBLK))):
            nb_blk = b1 - b0
            nd = int(needp[g, p]) + 64
            base = max(1, nd // (nb_blk * BLK))
            T8[g, b0:b1] = base
            extra = nd - base * nb_blk * BLK
            nb = max(0, -(-extra // BLK))
            for j in range(nb):
                T8[g, b0 + (g * 5 + j) % nb_blk] += 1

    for _attempt in range(8):
        capsA = (T8 * BLK).T[:SPLIT_B]        # [34, NG]
        capsB = (T8 * BLK).T[SPLIT_B:]
        blks = []
        ok = True
        for m in range(M):
            nd = np.arange(m * NPC_REAL, (m + 1) * NPC_REAL)
            bA = _assign_pool(deg[nd[:ASPLIT]], capsA, SPLIT_B)
            bB = _assign_pool(deg[nd[ASPLIT:]], capsB, NBLK - SPLIT_B)
            if bA is None or bB is None:
                ok = False
                break
            blks.append(np.concatenate([bA, bB + SPLIT_B]))
        if ok:
            break
        T8[:, (7 * _attempt) % NBLK] += 1     # grow every group, retry
    assert ok, "bin packing failed"

    # published node order
    pid = np.zeros(N, np.int64)
    slot_of = np.zeros(N, np.int64)
    for m in range(M):
        nd = np.arange(m * NPC_REAL, (m + 1) * NPC_REAL)
        b = blks[m]
        order = np.lexsort((nd, b))
        sl = np.zeros(NPC_REAL, np.int64)
        bb = b[order]
        newb = np.ones(NPC_REAL, bool)
        newb[1:] = bb[1:] != bb[:-1]
        starts = np.nonzero(newb)[0]
        runid = np.cumsum(newb) - 1
        sl[order] = np.arange(NPC_REAL) - starts[runid]
        assert sl.max() < BLK
        pid[nd] = m * NPC + b * BLK + sl
        slot_of[nd] = sl

    # src table row: A tables hold 4 rank slabs; B table holds all 8
    ppos = pid[src]
    prk = ppos // NPC
    pb = (ppos % NPC) // BLK
    psl = ppos % BLK
    isA = pb < SPLIT_B
    rowA = (prk % (M // 2)) * COLA + pb * BLK + psl
    rowB = prk * COLB + (pb - SPLIT_B) * BLK + psl
    srow = np.where(isA, rowA, rowB)
    assert (sreg == (~isA).astype(np.int64)).all()

    # group/tile/chunk structure
    groups = []            # (g, b, t0, T)
    tmeta = []             # per tile: (seg, b, st, sp)  seg = g
    t = 0
    for g in range(NG):
        for b in range(NBLK):
            tq = int(T8[g, b])
            groups.append((g, b, t, tq))
            for i in range(tq):
                tmeta.append((g, b, i == 0, i == tq - 1))
            t += tq
    TT = t
    chunks = []            # (t0, nt, g)
    gi = 0
    while gi < len(groups):
        g, b, t0, tq = groups[gi]
        nt = tq
        gj = gi + 1
        while (gj < len(groups) and groups[gj][0] == g
               and nt + groups[gj][3] <= CHUNK_T):
            nt += groups[gj][3]
            gj += 1
        chunks.append((t0, nt, g))
        gi = gj
    group_t0 = {(g, b): (t0, tq) for (g, b, t0, tq) in groups}

    # place edges
    bof = np.empty(len(dst), np.int64)
    for m in range(M):
        selm = core == m
        bof[selm] = blks[m][dst[selm] - m * NPC_REAL]
    key = (core * NG + sg) * NBLK + bof
    order = np.argsort(key, kind="stable")
    sk = key[order]
    new_run = np.ones(len(sk), bool)
    new_run[1:] = sk[1:] != sk[:-1]
    starts = np.nonzero(new_run)[0]
    runid = np.cumsum(new_run) - 1
    rank = np.arange(len(sk)) - starts[runid]
    t0g = np.array([group_t0[(g, b)][0] for g in range(NG)
                    for b in range(NBLK)]).reshape(NG, NBLK)
    pos = t0g[sg[order], bof[order]] * TILE_E + rank
    assert (rank < (T8 * BLK)[sg[order], bof[order]]).all(), "group overflow"

    kv_idx = np.zeros((M, TT * TILE_E), np.int64)
    dloc = np.full((M, TT * TILE_E), -1, np.int64)
    kv_idx[core[order], pos] = srow[order]
    dloc[core[order], pos] = slot_of[dst[order]]
    assert kv_idx.max() < 2 ** 15

    # trailing pads of each chunk -> idx -1 (stripped by the gather ucode)
    cnts = np.zeros((M, NG, NBLK), np.int64)
    np.add.at(cnts, (core, sg, bof), 1)
    pad_tiles = []
    for (t0, nt, g) in chunks:
        padt = 0
        for (gg, b, g0, tq) in groups:
            if gg == g and g0 + tq == t0 + nt:
                for m in range(M):
                    c = int(cnts[m, g, b])
                    lo = g0 * TILE_E + c
                    hi = (g0 + tq) * TILE_E
                    if lo < hi:
                        kv_idx[m, lo:hi] = -1
                    padt = max(padt, -(-(hi - lo) // TILE_E))
                break
        pad_tiles.append(padt)

    kv_w = _wrap_idx(kv_idx)

    # one-hot S / ST in fp8 (exact 0/1); S: [e, t*128+d], ST: [d, t*128+e]
    FP8NP = ml_dtypes.float8_e4m3fn
    S_in = np.zeros((M, 128, TT * TILE_E), FP8NP)
    ST_in = np.zeros((M, 128, TT * TILE_E), FP8NP)
    dl = dloc.reshape(M, TT, TILE_E)
    mm, tt_, pp_ = np.nonzero(dl >= 0)
    dv = dl[mm, tt_, pp_]
    S_in[mm, pp_, tt_ * TILE_E + dv] = 1.0
    ST_in[mm, dv, tt_ * TILE_E + pp_] = 1.0

    xT_pad = np.zeros((D_IN, NPAD), np.float32)
    xT_pad[:, pid] = np.asarray(x).T
    xT = xT_pad.astype(BNP)

    wt = {}
    for L in (1, 2, 3):
        W_all, b_all = weights[L]
        if L == 1:
            wt["W1"] = W_all.astype(BNP)
            wt["brep1"] = np.ascontiguousarray(
                np.tile(b_all[None, :], (128, 1)).astype(np.float32))
        else:
            wt[f"W{L}"] = np.concatenate(
                [W_all, b_all[None, :]], 0).astype(BNP)

    in_maps = []
    for m in range(M):
        im = dict(
            xT=np.ascontiguousarray(xT),
            kvidx=np.ascontiguousarray(kv_w[m]),
            S_in=np.ascontiguousarray(S_in[m]),
            ST_in=np.ascontiguousarray(ST_in[m]),
            **wt,
        )
        in_maps.append(im)

    meta = dict(TT=TT, tmeta=tmeta, chunks=chunks, T=T8, pid=pid,
                pad_tiles=pad_tiles)
    return in_maps, meta


def build_module(d, meta):
    TT, chunks, tmeta = meta["TT"], meta["chunks"], meta["tmeta"]
    pad_tiles = meta["pad_tiles"]
    M, NPC, NPAD, NBLK = d["M"], d["NPC"], d["NPAD"], d["NBLK"]
    D_IN, F, KV, WC, HID, H, C, RHSW = (
        d["D_IN"], d["F"], d["KV"], d["WC"], d["HID"], d["H"], d["C"],
        d["RHSW"])
    COLA, COLB, TABA, TABB = d["COLA"], d["COLB"], d["TABA"], d["TABB"]

    nc = bacc.Bacc("TRN2", target_bir_lowering=False, debug=False,
                   num_devices=M)
    inp = {}
    FP8 = mybir.dt.float8e4
    for name, shape, dt in [
        ("xT", [D_IN, NPAD], BF16), ("xoT", [D_IN, NPC], BF16),
        ("W1", [D_IN, WC], BF16), ("brep1", [128, WC], F32),
        ("W2", [HID + 1, WC], BF16), ("W3", [HID + 1, WC], BF16),
        ("kvidx", [128, TT * 8], I16),
        ("S_in", [128, TT * TILE_E], FP8), ("ST_in", [128, TT * TILE_E], FP8),
    ]:
        inp[name] = nc.dram_tensor(name, shape, dt, kind="ExternalInput")
    h_out = nc.dram_tensor("h_out", [NPC, HID], F32, kind="ExternalOutput")

    with tile.TileContext(nc) as tc:
        with tc.tile_pool(name="dram", bufs=1, space="DRAM") as dram, \
             tc.tile_pool(name="res", bufs=1) as res:
            tabs = [dram.tile([TABA, KV], BF16, name="tabA0"),
                    dram.tile([TABA, KV], BF16, name="tabA1"),
                    dram.tile([TABB, KV], BF16, name="tabB")]
            coll_inA = dram.tile([HID + 1, COLA], BF16)
            coll_outA = dram.tile([M * (HID + 1), COLA], BF16)
            coll_inB = dram.tile([HID + 1, COLB], BF16)
            coll_outB = dram.tile([M * (HID + 1), COLB], BF16)

            nc.gpsimd.load_library(library_config.mlp)

            W1_sb = res.tile([D_IN, WC], BF16)
            brep1_sb = res.tile([128, WC], F32)
            W2_sb = res.tile([HID + 1, WC], BF16)
            W3_sb = res.tile([HID + 1, WC], BF16)
            kvidx_sb = res.tile([128, TT * 8], I16)
            q_sb = res.tile([128, NBLK * F], BF16)
            s_sb = res.tile([128, NBLK * HID], F32)
            hTown = res.tile([HID + 1, NPC], BF16)
            partA = res.tile([128, NBLK * RHSW], F32)
            ident = res.tile([128, 128], F32)
            eps2 = res.tile([128, H], F32)

            for sb, t in ((W1_sb, "W1"), (brep1_sb, "brep1"), (W2_sb, "W2"),
                          (W3_sb, "W3"), (kvidx_sb, "kvidx")):
                nc.sync.dma_start(sb[:], inp[t].ap())
            make_identity(nc, ident[:])
            nc.vector.memset(hTown[HID:HID + 1, :], 1.0)
            nc.vector.memset(eps2[:], H * 1e-16)

            with tc.tile_pool(name="pa", bufs=4) as pa, \
                 tc.tile_pool(name="pap", bufs=2, space="PSUM") as pap, \
                 tc.tile_pool(name="pb", bufs=3) as pb, \
                 tc.tile_pool(name="pb1", bufs=2) as pb1, \
                 tc.tile_pool(name="pbp", bufs=3, space="PSUM") as pbp, \
                 tc.tile_pool(name="qep", bufs=2, space="PSUM") as qep, \
                 tc.tile_pool(name="epp", bufs=1, space="PSUM") as epp, \
                 tc.tile_pool(name="ep", bufs=2) as ep:
              for layer in (1, 2, 3):
                W_sb = {1: W1_sb, 2: W2_sb, 3: W3_sb}[layer]

                # ---------- Phase A ----------
                if True:
                    def emit_kv(tab, row0, ntl, la):
                        kvst = pa.tile([128, PA_CHUNK * KV], BF16, tag="kvst")
                        for t in range(ntl):
                            ps = pap.tile([128, KV], F32, tag="pskv",
                                          name="pskv")
                            dstp = kvst[:, t * KV:(t + 1) * KV]
                            nc.tensor.matmul(ps[:],
                                             la[:, t * 128:(t + 1) * 128],
                                             W_sb[:, 0:KV],
                                             start=True, stop=True)
                            nc.scalar.copy(dstp, ps[:])
                        if layer == 1:
                            nc.vector.tensor_tensor(
                                kvst[:, 0:ntl * KV].rearrange(
                                    "p (t e) -> p t e", e=KV),
                                kvst[:, 0:ntl * KV].rearrange(
                                    "p (t e) -> p t e", e=KV),
                                brep1_sb[:, 0:KV].rearrange(
                                    "p (t e) -> p t e", t=1).to_broadcast(
                                        [128, ntl, KV]),
                                op=OP.add)
                        nc.sync.dma_start(
                            tab[row0:row0 + ntl * 128, :].rearrange(
                                "(t p) e -> p t e", p=128),
                            kvst[:, 0:ntl * KV].rearrange(
                                "p (t e) -> p t e", e=KV))

                    def emit_region(reg):
                        colr, regt = ((COLA, COLA // 128) if reg == 0
                                      else (COLB, COLB // 128))
                        for rk in range(M):
                            ch = 0
                            while ch < regt:
                                ntl = min(PA_CHUNK, regt - ch)
                                if layer == 1:
                                    la = pa.tile([D_IN, PA_CHUNK * 128],
                                                 BF16, tag="la")
                                    c0 = rk * NPC + reg * COLA + ch * 128
                                    nc.sync.dma_start(
                                        la[:, 0:ntl * 128],
                                        inp["xT"].ap()[:, c0:c0 + ntl * 128])
                                else:
                                    la = pa.tile([HID + 1, PA_CHUNK * 128],
                                                 BF16, tag="la")
                                    reg_t = (coll_outA if reg == 0
                                             else coll_outB)
                                    nc.sync.dma_start(
                                        la[:, 0:ntl * 128],
                                        reg_t[rk * (HID + 1):
                                              (rk + 1) * (HID + 1),
                                              ch * 128:(ch + ntl) * 128])
                                if reg == 0:
                                    tab = tabs[rk // (M // 2)]
                                    row0 = (rk % (M // 2)) * colr + ch * 128
                                else:
                                    tab = tabs[2]
                                    row0 = rk * colr + ch * 128
                                emit_kv(tab, row0, ntl, la)
                                ch += ntl
                            if reg == 0 and rk == M // 2 - 1:
                                emit_qs()

                    def emit_qs():
                        for ch in range(0, NBLK, PA_CHUNK):
                            ntl = min(PA_CHUNK, NBLK - ch)
                            if layer == 1:
                                la = pa.tile([D_IN, PA_CHUNK * 128], BF16,
                                             tag="la")
                                # own cols inside xT (device id 0 layout is
                                # identical across cores; per-core data maps
                                # provide the right xT, and own cols are at
                                # device-rank offset) -- use replica-specific
                                # xoT slice of xT instead:
                                c0 = ch * 128
                                nc.sync.dma_start(
                                    la[:, 0:ntl * 128],
                                    inp["xoT"].ap()[:, c0:c0 + ntl * 128])
                            for t in range(ntl):
                                gt = ch + t
                                ps = pap.tile([128, KV], F32,
                                              tag="pskv", name="psqs")
                                if layer == 1:
                                    nc.tensor.matmul(
                                        ps[:, 0:F + HID],
                                        la[:, t * 128:(t + 1) * 128],
                                        W1_sb[:, KV:WC], start=True, stop=True)
                                else:
                                    nc.tensor.matmul(
                                        ps[:, 0:F + HID],
                                        hTown[:, gt * 128:(gt + 1) * 128],
                                        W_sb[:, KV:WC], start=True, stop=True)
                                nc.vector.tensor_copy(
                                    q_sb[:, gt * F:(gt + 1) * F],
                                    ps[:, 0:F])
                                nc.scalar.copy(
                                    s_sb[:, gt * HID:(gt + 1) * HID],
                                    ps[:, F:F + HID])
                            if layer == 1:
                                nc.vector.tensor_tensor(
                                    q_sb[:, ch * F:(ch + ntl) * F].rearrange(
                                        "p (t e) -> p t e", e=F),
                                    q_sb[:, ch * F:(ch + ntl) * F].rearrange(
                                        "p (t e) -> p t e", e=F),
                                    brep1_sb[:, KV:KV + F].rearrange(
                                        "p (t e) -> p t e", t=1).to_broadcast(
                                            [128, ntl, F]),
                                    op=OP.add)
                                nc.vector.tensor_tensor(
                                    s_sb[:, ch * HID:(ch + ntl) * HID
                                         ].rearrange("p (t e) -> p t e", e=HID),
                                    s_sb[:, ch * HID:(ch + ntl) * HID
                                         ].rearrange("p (t e) -> p t e", e=HID),
                                    brep1_sb[:, KV + F:WC].rearrange(
                                        "p (t e) -> p t e", t=1).to_broadcast(
                                            [128, ntl, HID]),
                                    op=OP.add)

                    emit_region(0)
                    emit_region(1)

                # ---------- Phase B ----------
                if True:
                    psum_g = None
                    for cj, (t0, nt, g) in enumerate(chunks):
                        n = nt * TILE_E
                        tab = tabs[g]
                        kvg = pb.tile([128, CHUNK_T, KV], BF16, tag="kvg")
                        rhs = pb.tile([128, CHUNK_T, RHSW], BF16, tag="rhs")
                        Sg = pb1.tile([128, CHUNK_T, 128], FP8, tag="Sg")
                        STg = pb1.tile([128, CHUNK_T, 128], FP8, tag="STg")
                        prod = pb1.tile([128, CHUNK_T * F], BF16, tag="prod")
                        alph = pb1.tile([128, CHUNK_T * H], F32, tag="alph")

                        if layer == 1 and cj < 3:
                            nc.vector.memset(
                                kvg[:].rearrange("p a b -> p (a b)"), 0.0)
                        nc.gpsimd.dma_gather(
                            out_ap=kvg[:, 0:nt, :], in_ap=tab[:],
                            idxs_ap=kvidx_sb[:, t0 * 8:t0 * 8 + nt * 8],
                            num_idxs=n, num_idxs_reg=n, elem_size=KV,
                            single_packet=False)
                        nc.sync.dma_start(
                            Sg[:, 0:nt, :].rearrange("p a b -> p (a b)"),
                            inp["S_in"].ap()[:, t0 * TILE_E:t0 * TILE_E + n])
                        nc.sync.dma_start(
                            STg[:, 0:nt, :].rearrange("p a b -> p (a b)"),
                            inp["ST_in"].ap()[:, t0 * TILE_E:t0 * TILE_E + n])

                        i = 0
                        while i < nt:
                            ng = min(QE_G, nt - i)
                            qeg = qep.tile([128, QE_G * F], F32, name="qeg",
                                           tag="qeg")
                            for j in range(ng):
                                b = tmeta[t0 + i + j][1]
                                nc.tensor.matmul(
                                    qeg[:, j * F:(j + 1) * F],
                                    STg[:, i + j, :],
                                    q_sb[:, b * F:(b + 1) * F],
                                    start=True, stop=True)
                            nc.vector.tensor_tensor(
                                out=prod[:, i * F:(i + ng) * F].rearrange(
                                    "p (t f) -> p t f", f=F),
                                in0=qeg[:, 0:ng * F].rearrange(
                                    "p (t f) -> p t f", f=F),
                                in1=kvg[:, i:i + ng, 0:F],
                                op=OP.mult)
                            i += ng
                        nc.vector.reduce_sum(
                            alph[:, 0:nt * H].rearrange("p (t h) -> p t h",
                                                        h=H),
                            prod[:, 0:nt * F].rearrange(
                                "p (t h c) -> p t h c", h=H, c=C),
                            axis=mybir.AxisListType.X)
                        nc.scalar.activation(
                            rhs[:, 0:nt, 0:H],
                            alph[:, 0:nt * H].rearrange("p (t h) -> p t h",
                                                        h=H),
                            AF.Exp)
                        nc.vector.tensor_tensor(
                            out=rhs[:, 0:nt, H:RHSW].rearrange(
                                "p t (h c) -> p t h c", c=C),
                            in0=kvg[:, 0:nt, F:KV].rearrange(
                                "p t (h c) -> p t h c", c=C),
                            in1=rhs[:, 0:nt, 0:H].to_broadcast(
                                [128, nt, H, C]),
                            op=OP.mult)

                        for i in range(nt):
                            gg, b, st, sp = tmeta[t0 + i]
                            if st:
                                psum_g = pbp.tile([128, RHSW], F32,
                                                  name="pblk", tag="pblk")
                            nc.tensor.matmul(
                                psum_g[:], Sg[:, i, :], rhs[:, i, :],
                                start=st, stop=sp)
                            if not sp:
                                continue
                            pa_sl = partA[:, b * RHSW:(b + 1) * RHSW]
                            if gg == 0:
                                nc.vector.tensor_copy(pa_sl, psum_g[:])
                                continue
                            if gg < NG - 1:
                                nc.vector.tensor_tensor(pa_sl, psum_g[:],
                                                        pa_sl, op=OP.add)
                                continue
                            # ---- epilogue for block b ----
                            tot = ep.tile([128, RHSW], F32, tag="tot")
                            nc.vector.tensor_tensor(tot[:], psum_g[:], pa_sl,
                                                    op=OP.add)
                            rec = ep.tile([128, H], F32, tag="rec")
                            nc.vector.scalar_tensor_tensor(
                                out=rec[:], in0=tot[:, 0:H], scalar=float(H),
                                in1=eps2[:], op0=OP.mult, op1=OP.add)
                            nc.vector.reciprocal(rec[:], rec[:])
                            m0 = ep.tile([128, C], F32, tag="m0")
                            nc.vector.scalar_tensor_tensor(
                                out=m0[:], in0=tot[:, H:H + C],
                                scalar=rec[:, 0:1],
                                in1=s_sb[:, b * HID:(b + 1) * HID],
                                op0=OP.mult, op1=OP.add)
                            hp2 = ep.tile([128, HID], F32, tag="hp2")
                            nc.vector.scalar_tensor_tensor(
                                out=hp2[:], in0=tot[:, H + C:H + 2 * C],
                                scalar=rec[:, 1:2], in1=m0[:],
                                op0=OP.mult, op1=OP.add)
                            hblk = ep.tile([128, HID], F32, tag="hblk")
                            nc.scalar.activation(hblk[:], hp2[:], AF.Relu)
                            if layer < 3:
                                pst = epp.tile([HID, 128], F32)
                                nc.tensor.transpose(pst[:], hblk[:], ident[:])
                                nc.vector.tensor_copy(
                                    hTown[0:HID, b * 128:(b + 1) * 128],
                                    pst[:])
                                if b == SPLIT_B - 1:
                                    nc.sync.dma_start(coll_inA[:, :],
                                                      hTown[:, 0:COLA])
                                    nc.gpsimd.collective_compute(
                                        "AllGather", OP.bypass,
                                        ins=[coll_inA.opt()],
                                        outs=[coll_outA.opt()],
                                        replica_groups=[list(range(M))])
                            else:
                                nc.sync.dma_start(
                                    h_out.ap()[b * 128:(b + 1) * 128, :],
                                    hblk[:])

                if layer < 3:
                    nc.sync.dma_start(coll_inB[:, :], hTown[:, COLA:])
                    nc.gpsimd.collective_compute(
                        "AllGather", OP.bypass,
                        ins=[coll_inB.opt()], outs=[coll_outB.opt()],
                        replica_groups=[list(range(M))])
    nc.compile()
    return nc


# ---------------- public entry ----------------
_CACHE = {}


def _weights_from_inputs(inputs, d):
    # packed column order: k | v | q | s ; q pre-scaled by 1/sqrt(C)
    sc = 1.0 / np.sqrt(d["C"])
    wt = {}
    for L in (1, 2, 3):
        Ws, bs = [], []
        for nm in ("k", "v", "q", "s"):
            W = np.asarray(inputs[f"W{L}{nm}"], np.float32)
            b = np.asarray(inputs[f"b{L}{nm}"], np.float32)
            if nm == "q":
                W = W * sc
                b = b * sc
            Ws.append(W)
            bs.append(b)
        wt[L] = (np.concatenate(Ws, axis=1), np.concatenate(bs))
    return wt


def _install_ntff_shim():
    import types
    if "antenv.axon_hooks" in sys.modules:
        return
    try:
        from trn_agent_boot.trn_boot import _ntff_profile_via_ctypes
        hook = _ntff_profile_via_ctypes("/opt/axon/libaxon_pjrt.so")
    except Exception:
        hook = None
    mod = types.ModuleType("antenv.axon_hooks")
    mod.get_axon_ntff_profile_hook = lambda: hook
    mod.set_axon_ntff_profile_hook = lambda h: None
    sys.modules["antenv.axon_hooks"] = mod
    try:
        import antenv
        antenv.axon_hooks = mod
    except Exception:
        pass


def run(inputs, cfg=SPEC, trace=False):
    d = _derive(cfg)
    wt = _weights_from_inputs(inputs, d)
    in_maps, meta = _prep(inputs["x"], inputs["edge_index"], wt, d)
    for m in range(d["M"]):
        in_maps[m]["xoT"] = np.ascontiguousarray(
            in_maps[m]["xT"][:, m * d["NPC"]:(m + 1) * d["NPC"]])
    key = (tuple(sorted(cfg.items())), meta["TT"],
           tuple(meta["T"].flatten().tolist()))
    if key not in _CACHE:
        _CACHE[key] = build_module(d, meta)
    nc = _CACHE[key]
    if trace:
        _install_ntff_shim()
    res = bass_utils.run_bass_kernel_spmd(
        nc, in_maps, core_ids=list(range(d["M"])), trace=trace)
    pid = meta["pid"]
    N, NPC = d["N"], d["NPC"]
    full = np.empty((N, d["HID"]), np.float32)
    for m in range(d["M"]):
        sel = np.arange(m * d["NPC_REAL"], (m + 1) * d["NPC_REAL"])
        full[sel] = res.results[m]["h_out"][pid[sel] - m * NPC]
    return full, res


def kernel(**inputs) -> np.ndarray:
    trace = bool(os.environ.get("KERNEL_TRACE"))
    full, res = run(inputs, SPEC, trace=trace)
    if trace and res.exec_time_ns is not None:
        print(f"HW exec time: {res.exec_time_ns} ns")
    return full


# revision 32
# speedup vs baseline: 1.1515x; 1.0770x over previous
"""Trainium2 Bass kernel for a 3-layer TransformerConv GNN (nn_EncoderTransformerConv).

Strategy (8 NeuronCores, SPMD, identical instruction stream per core):
  - Nodes are partitioned across cores; each core's 6250 dst nodes are
    re-grouped (host bin-packing) into 49 blocks of 128 so that every
    (src-group, dst-block) edge group exactly fills its statically-sized
    tile budget -> the Q7 dma_gather scans a near-minimal number of index
    slots (the gather ucode at ~7.75 ns/slot is the kernel's critical
    resource).
  - Edges are grouped by 8 src-groups = (src quarter) x (src A/B region),
    where the A region of every core is the block range covered by the
    FIRST AllGather collective.  All A-region edge work is scheduled before
    any B-region work, so the second (layer-end) collective and the next
    layer's B-table builds overlap the A-region edge phase.
  - Per layer:
      Phase A  : every core computes the k|v projections for ALL nodes
                 (replicated dense bf16 matmuls) into 8 DRAM tables
                 (per quarter x region; <=8704 rows -> int16 indices); also
                 q (pre-scaled by 1/sqrt(C)) and root-skip s for OWN nodes.
      Phase B  : per chunk: dma_gather of kv[src] rows; one-hot S / ST
                 matrices generated ON-CHIP with DVE is_equal compares from
                 compact int8 dloc tensors (no one-hot DMA); q broadcast to
                 edges via ST matmul (4 tiles share one PSUM bank so the
                 q*k product is one DVE op per 4 tiles); alpha reduce; exp;
                 ex*v; one-hot S matmul reduces into PSUM per group,
                 accumulated across the 8 src-groups in an SBUF partial.
      Epilogue : normalize, mean heads, add skip, relu.
  - Pad slots carry dloc=-1 (their one-hot columns are all-zero, so they are
    dropped by the S matmul); chunk-trailing pads carry index -1, which the
    gather ucode strips.  SBUF gather buffers are memset once (layer 1) so
    never-written pad rows hold finite values.
"""
import os
import sys

sys.path.insert(0, "/opt/trn_rl_repo")

import ml_dtypes
import numpy as np

import concourse.bass as bass
import concourse.bacc as bacc
import concourse.mybir as mybir
import concourse.tile as tile
from concourse import bass_utils, library_config
from concourse.masks import make_identity

F32 = mybir.dt.float32
BF16 = mybir.dt.bfloat16
I16 = mybir.dt.int16
I8 = mybir.dt.int8
AF = mybir.ActivationFunctionType
OP = mybir.AluOpType
BNP = ml_dtypes.bfloat16

# ----- problem dims (hardcoded per spec) -----
SPEC = dict(N=50000, E=800000, D_IN=128, HID=64, H=2, M=8)
TILE_E = 128          # edges per tile
CHUNK_T = 24          # max tiles per edge-phase chunk (whole groups)
BLK = 128             # dst nodes per block
NG = 3                # src groups: A ranks 0-3 | A ranks 4-7 | B all ranks
SPLIT_B = 26          # blocks covered by first collective (A region)
PA_CHUNK = 7          # node tiles per phase-A chunk
QE_G = 4              # qe tiles sharing one PSUM bank


def _derive(cfg):
    d = dict(cfg)
    d["C"] = d["HID"]
    d["F"] = d["H"] * d["C"]            # 128 = q/k/v width
    d["WC"] = 3 * d["F"] + d["HID"]     # 448 packed k|v|q|s
    d["KV"] = 2 * d["F"]                # 256
    d["NPC_REAL"] = d["N"] // d["M"]
    d["NBLK"] = -(-d["NPC_REAL"] // BLK)
    d["NPC"] = d["NBLK"] * BLK
    d["NPAD"] = d["M"] * d["NPC"]
    d["RHSW"] = d["H"] + d["F"]         # 130 = denom cols + exv cols
    d["COLA"] = SPLIT_B * BLK           # A-region cols per core
    d["COLB"] = d["NPC"] - d["COLA"]
    d["TABA"] = (d["M"] // 2) * d["COLA"]   # rows per A table (rank quad)
    d["TABB"] = d["M"] * d["COLB"]          # rows of the single B table
    assert d["TABA"] < 2 ** 15 and d["TABB"] < 2 ** 15
    # original-id split of each core's nodes into A/B pools
    d["ASPLIT"] = d["NPC_REAL"] - (d["COLB"] - 15)
    return d


def _wrap_idx(a):
    """[M, n] int -> wrapped idx layout [M, 128, n//16]."""
    Mn, n = a.shape
    w = a.reshape(Mn, n // 16, 16).transpose(0, 2, 1)
    return np.ascontiguousarray(np.tile(w, (1, 8, 1))).astype(np.int16)


def _assign_pool(deg, caps, nblk):
    """Nodes (rows of deg [n, G]) -> nblk blocks of <=128 nodes with
    per-(block, g) loads <= caps [nblk, G].  Soft greedy, then swap-repair
    local search.  Returns blk or None."""
    n, G = deg.shape
    rng = np.random.default_rng(0)
    order = np.argsort(-deg.sum(1), kind="stable")
    L = np.zeros((nblk, G), np.int64)
    cnt = np.zeros(nblk, np.int64)
    blk = np.full(n, -1, np.int64)
    capf = caps.astype(np.float64)
    for i in order:
        need = deg[i]
        ok = cnt < BLK
        ratio = ((L + need) / capf).max(axis=1)
        ratio[~ok] = 1e18
        b = int(np.argmin(ratio))
        blk[i] = b
        L[b] += need
        cnt[b] += 1
    # swap-repair
    for _ in range(40000):
        viol = L - caps
        vb, vg = np.unravel_index(np.argmax(viol), viol.shape)
        if viol[vb, vg] <= 0:
            return blk
        in_b = np.nonzero(blk == vb)[0]
        # pick a heavy-in-g node of vb, swap with a light node elsewhere
        cand_i = in_b[np.argsort(-deg[in_b, vg])[:8]]
        best = None
        for i in cand_i:
            di = deg[i]
            out = np.nonzero(blk != vb)[0]
            if len(out) > 3000:
                out = rng.choice(out, 3000, replace=False)
            dj = deg[out]
            b2 = blk[out]
            # new loads after swap
            nb = L[vb] - di + dj                  # [k, G]
            n2 = L[b2] + di - dj
            v_new = (np.maximum(nb - caps[vb], 0).sum(1)
                     + np.maximum(n2 - caps[b2], 0).sum(1)
                     + np.maximum(L - caps, 0).sum()
                     - np.maximum(L[vb] - caps[vb], 0).sum()
                     - np.maximum(L[b2] - caps[b2], 0).sum(1))
            j = int(np.argmin(v_new))
            if best is None or v_new[j] < best[0]:
                best = (v_new[j], i, out[j])
        cur = np.maximum(L - caps, 0).sum()
        if best is None or best[0] >= cur:
            return None
        _, i, j = best
        bi, bj = blk[i], blk[j]
        L[bi] += deg[j] - deg[i]
        L[bj] += deg[i] - deg[j]
        blk[i], blk[j] = bj, bi
    return None


def _prep(x, edge_index, weights, d):
    M, NPC_REAL, NPC, NPAD, NBLK = (
        d["M"], d["NPC_REAL"], d["NPC"], d["NPAD"], d["NBLK"])
    N, D_IN, ASPLIT, COLA, COLB = (
        d["N"], d["D_IN"], d["ASPLIT"], d["COLA"], d["COLB"])

    src = np.asarray(edge_index[0]).astype(np.int64)
    dst = np.asarray(edge_index[1]).astype(np.int64)
    core = dst // NPC_REAL
    scc = src // NPC_REAL
    sreg = ((src - scc * NPC_REAL) >= ASPLIT).astype(np.int64)
    sg = np.where(sreg == 1, 2, scc // (M // 2))   # A0 | A1 | B

    deg = np.zeros((N, NG), np.int64)
    np.add.at(deg, (dst, sg), 1)

    # per-(core, group, dst-pool) edge counts -> per-pool cap budgets
    dpool = ((dst - core * NPC_REAL) >= ASPLIT).astype(np.int64)
    Emgp = np.zeros((M, NG, 2), np.int64)
    np.add.at(Emgp, (core, sg, dpool), 1)
    needp = Emgp.max(axis=0)                  # [NG, 2]

    # per-(group, block) tile budgets: minimal per-pool total, rotated bumps
    T8 = np.zeros((NG, NBLK), np.int64)
    for g in range(NG):
        for p, (b0, b1) in enumerate(((0, SPLIT_B), (SPLIT_B, NBLK))):
            nb_blk = b1 - b0
            nd = int(needp[g, p]) + 64
            base = max(1, nd // (nb_blk * BLK))
            T8[g, b0:b1] = base
            extra = nd - base * nb_blk * BLK
            nb = max(0, -(-extra // BLK))
            for j in range(nb):
                T8[g, b0 + (g * 5 + j) % nb_blk] += 1

    for _attempt in range(8):
        capsA = (T8 * BLK).T[:SPLIT_B]        # [34, NG]
        capsB = (T8 * BLK).T[SPLIT_B:]
        blks = []
        ok = True
        for m in range(M):
            nd = np.arange(m * NPC_REAL, (m + 1) * NPC_REAL)
            bA = _assign_pool(deg[nd[:ASPLIT]], capsA, SPLIT_B)
            bB = _assign_pool(deg[nd[ASPLIT:]], capsB, NBLK - SPLIT_B)
            if bA is None or bB is None:
                ok = False
                break
            blks.append(np.concatenate([bA, bB + SPLIT_B]))
        if ok:
            break
        T8[:, (7 * _attempt) % NBLK] += 1     # grow every group, retry
    assert ok, "bin packing failed"

    # published node order
    pid = np.zeros(N, np.int64)
    slot_of = np.zeros(N, np.int64)
    for m in range(M):
        nd = np.arange(m * NPC_REAL, (m + 1) * NPC_REAL)
        b = blks[m]
        order = np.lexsort((nd, b))
        sl = np.zeros(NPC_REAL, np.int64)
        bb = b[order]
        newb = np.ones(NPC_REAL, bool)
        newb[1:] = bb[1:] != bb[:-1]
        starts = np.nonzero(newb)[0]
        runid = np.cumsum(newb) - 1
        sl[order] = np.arange(NPC_REAL) - starts[runid]
        assert sl.max() < BLK
        pid[nd] = m * NPC + b * BLK + sl
        slot_of[nd] = sl

    # src table row: A tables hold 4 rank slabs; B table holds all 8
    ppos = pid[src]
    prk = ppos // NPC
    pb = (ppos % NPC) // BLK
    psl = ppos % BLK
    isA = pb < SPLIT_B
    rowA = (prk % (M // 2)) * COLA + pb * BLK + psl
    rowB = prk * COLB + (pb - SPLIT_B) * BLK + psl
    srow = np.where(isA, rowA, rowB)
    assert (sreg == (~isA).astype(np.int64)).all()

    # group/tile/chunk structure
    groups = []            # (g, b, t0, T)
    tmeta = []             # per tile: (seg, b, st, sp)  seg = g
    t = 0
    for g in range(NG):
        for b in range(NBLK):
            tq = int(T8[g, b])
            groups.append((g, b, t, tq))
            for i in range(tq):
                tmeta.append((g, b, i == 0, i == tq - 1))
            t += tq
    TT = t
    chunks = []            # (t0, nt, g)
    gi = 0
    while gi < len(groups):
        g, b, t0, tq = groups[gi]
        nt = tq
        gj = gi + 1
        while (gj < len(groups) and groups[gj][0] == g
               and nt + groups[gj][3] <= CHUNK_T):
            nt += groups[gj][3]
            gj += 1
        chunks.append((t0, nt, g))
        gi = gj
    group_t0 = {(g, b): (t0, tq) for (g, b, t0, tq) in groups}

    # place edges
    bof = np.empty(len(dst), np.int64)
    for m in range(M):
        selm = core == m
        bof[selm] = blks[m][dst[selm] - m * NPC_REAL]
    key = (core * NG + sg) * NBLK + bof
    order = np.argsort(key, kind="stable")
    sk = key[order]
    new_run = np.ones(len(sk), bool)
    new_run[1:] = sk[1:] != sk[:-1]
    starts = np.nonzero(new_run)[0]
    runid = np.cumsum(new_run) - 1
    rank = np.arange(len(sk)) - starts[runid]
    t0g = np.array([group_t0[(g, b)][0] for g in range(NG)
                    for b in range(NBLK)]).reshape(NG, NBLK)
    pos = t0g[sg[order], bof[order]] * TILE_E + rank
    assert (rank < (T8 * BLK)[sg[order], bof[order]]).all(), "group overflow"

    kv_idx = np.zeros((M, TT * TILE_E), np.int64)
    dloc = np.full((M, TT * TILE_E), -1, np.int64)
    kv_idx[core[order], pos] = srow[order]
    dloc[core[order], pos] = slot_of[dst[order]]
    assert kv_idx.max() < 2 ** 15

    # trailing pads of each chunk -> idx -1 (stripped by the gather ucode)
    cnts = np.zeros((M, NG, NBLK), np.int64)
    np.add.at(cnts, (core, sg, bof), 1)
    pad_tiles = []
    for (t0, nt, g) in chunks:
        padt = 0
        for (gg, b, g0, tq) in groups:
            if gg == g and g0 + tq == t0 + nt:
                for m in range(M):
                    c = int(cnts[m, g, b])
                    lo = g0 * TILE_E + c
                    hi = (g0 + tq) * TILE_E
                    if lo < hi:
                        kv_idx[m, lo:hi] = -1
                    padt = max(padt, -(-(hi - lo) // TILE_E))
                break
        pad_tiles.append(padt)

    kv_w = _wrap_idx(kv_idx)

    # one-hot S / ST in fp8 (exact 0/1); S: [e, t*128+d], ST: [d, t*128+e]
    FP8NP = ml_dtypes.float8_e4m3fn
    S_in = np.zeros((M, 128, TT * TILE_E), FP8NP)
    ST_in = np.zeros((M, 128, TT * TILE_E), FP8NP)
    dl = dloc.reshape(M, TT, TILE_E)
    mm, tt_, pp_ = np.nonzero(dl >= 0)
    dv = dl[mm, tt_, pp_]
    S_in[mm, pp_, tt_ * TILE_E + dv] = 1.0
    ST_in[mm, dv, tt_ * TILE_E + pp_] = 1.0

    xT_pad = np.zeros((D_IN, NPAD), np.float32)
    xT_pad[:, pid] = np.asarray(x).T
    xT = xT_pad.astype(BNP)

    wt = {}
    for L in (1, 2, 3):
        W_all, b_all = weights[L]
        if L == 1:
            wt["W1"] = W_all.astype(BNP)
            wt["brep1"] = np.ascontiguousarray(
                np.tile(b_all[None, :], (128, 1)).astype(np.float32))
        else:
            wt[f"W{L}"] = np.concatenate(
                [W_all, b_all[None, :]], 0).astype(BNP)

    in_maps = []
    for m in range(M):
        im = dict(
            xT=np.ascontiguousarray(xT),
            kvidx=np.ascontiguousarray(kv_w[m]),
            S_in=np.ascontiguousarray(S_in[m]),
            ST_in=np.ascontiguousarray(ST_in[m]),
            **wt,
        )
        in_maps.append(im)

    meta = dict(TT=TT, tmeta=tmeta, chunks=chunks, T=T8, pid=pid,
                pad_tiles=pad_tiles)
    return in_maps, meta


def build_module(d, meta):
    TT, chunks, tmeta = meta["TT"], meta["chunks"], meta["tmeta"]
    pad_tiles = meta["pad_tiles"]
    M, NPC, NPAD, NBLK = d["M"], d["NPC"], d["NPAD"], d["NBLK"]
    D_IN, F, KV, WC, HID, H, C, RHSW = (
        d["D_IN"], d["F"], d["KV"], d["WC"], d["HID"], d["H"], d["C"],
        d["RHSW"])
    COLA, COLB, TABA, TABB = d["COLA"], d["COLB"], d["TABA"], d["TABB"]

    nc = bacc.Bacc("TRN2", target_bir_lowering=False, debug=False,
                   num_devices=M)
    inp = {}
    FP8 = mybir.dt.float8e4
    for name, shape, dt in [
        ("xT", [D_IN, NPAD], BF16), ("xoT", [D_IN, NPC], BF16),
        ("W1", [D_IN, WC], BF16), ("brep1", [128, WC], F32),
        ("W2", [HID + 1, WC], BF16), ("W3", [HID + 1, WC], BF16),
        ("kvidx", [128, TT * 8], I16),
        ("S_in", [128, TT * TILE_E], FP8), ("ST_in", [128, TT * TILE_E], FP8),
    ]:
        inp[name] = nc.dram_tensor(name, shape, dt, kind="ExternalInput")
    h_out = nc.dram_tensor("h_out", [NPC, HID], F32, kind="ExternalOutput")

    with tile.TileContext(nc) as tc:
        with tc.tile_pool(name="dram", bufs=1, space="DRAM") as dram, \
             tc.tile_pool(name="res", bufs=1) as res:
            tabs = [dram.tile([TABA, KV], BF16, name="tabA0"),
                    dram.tile([TABA, KV], BF16, name="tabA1"),
                    dram.tile([TABB, KV], BF16, name="tabB")]
            coll_inA = dram.tile([HID + 1, COLA], BF16)
            coll_outA = dram.tile([M * (HID + 1), COLA], BF16)
            coll_inB = dram.tile([HID + 1, COLB], BF16)
            coll_outB = dram.tile([M * (HID + 1), COLB], BF16)

            nc.gpsimd.load_library(library_config.mlp)

            W1_sb = res.tile([D_IN, WC], BF16)
            brep1_sb = res.tile([128, WC], F32)
            W2_sb = res.tile([HID + 1, WC], BF16)
            W3_sb = res.tile([HID + 1, WC], BF16)
            kvidx_sb = res.tile([128, TT * 8], I16)
            q_sb = res.tile([128, NBLK * F], BF16)
            s_sb = res.tile([128, NBLK * HID], F32)
            hTown = res.tile([HID + 1, NPC], BF16)
            partA = res.tile([128, NBLK * RHSW], F32)
            ident = res.tile([128, 128], F32)
            eps2 = res.tile([128, H], F32)

            for sb, t in ((W1_sb, "W1"), (brep1_sb, "brep1"), (W2_sb, "W2"),
                          (W3_sb, "W3"), (kvidx_sb, "kvidx")):
                nc.sync.dma_start(sb[:], inp[t].ap())
            make_identity(nc, ident[:])
            nc.vector.memset(hTown[HID:HID + 1, :], 1.0)
            nc.vector.memset(eps2[:], H * 1e-16)

            with tc.tile_pool(name="pa", bufs=4) as pa, \
                 tc.tile_pool(name="pap", bufs=2, space="PSUM") as pap, \
                 tc.tile_pool(name="pb", bufs=3) as pb, \
                 tc.tile_pool(name="pb1", bufs=2) as pb1, \
                 tc.tile_pool(name="pbp", bufs=3, space="PSUM") as pbp, \
                 tc.tile_pool(name="qep", bufs=2, space="PSUM") as qep, \
                 tc.tile_pool(name="epp", bufs=1, space="PSUM") as epp, \
                 tc.tile_pool(name="ep", bufs=2) as ep:
              for layer in (1, 2, 3):
                W_sb = {1: W1_sb, 2: W2_sb, 3: W3_sb}[layer]

                # ---------- Phase A (emission helpers; woven into B) ----
                if True:
                    def emit_kv(tab, row0, ntl, la):
                        kvst = pa.tile([128, PA_CHUNK * KV], BF16, tag="kvst")
                        for t in range(ntl):
                            ps = pap.tile([128, KV], F32, tag="pskv",
                                          name="pskv")
                            dstp = kvst[:, t * KV:(t + 1) * KV]
                            nc.tensor.matmul(ps[:],
                                             la[:, t * 128:(t + 1) * 128],
                                             W_sb[:, 0:KV],
                                             start=True, stop=True)
                            nc.scalar.copy(dstp, ps[:])
                        if layer == 1:
                            nc.vector.tensor_tensor(
                                kvst[:, 0:ntl * KV].rearrange(
                                    "p (t e) -> p t e", e=KV),
                                kvst[:, 0:ntl * KV].rearrange(
                                    "p (t e) -> p t e", e=KV),
                                brep1_sb[:, 0:KV].rearrange(
                                    "p (t e) -> p t e", t=1).to_broadcast(
                                        [128, ntl, KV]),
                                op=OP.add)
                        nc.sync.dma_start(
                            tab[row0:row0 + ntl * 128, :].rearrange(
                                "(t p) e -> p t e", p=128),
                            kvst[:, 0:ntl * KV].rearrange(
                                "p (t e) -> p t e", e=KV))

                    def pa_item(lyr, reg, rk, ch, ntl):
                        # one phase-A chunk for layer `lyr`
                        colr = COLA if reg == 0 else COLB
                        if lyr == 1:
                            la = pa.tile([D_IN, PA_CHUNK * 128], BF16,
                                         tag="la")
                            c0 = rk * NPC + reg * COLA + ch * 128
                            nc.sync.dma_start(
                                la[:, 0:ntl * 128],
                                inp["xT"].ap()[:, c0:c0 + ntl * 128])
                        else:
                            la = pa.tile([HID + 1, PA_CHUNK * 128], BF16,
                                         tag="la")
                            reg_t = coll_outA if reg == 0 else coll_outB
                            nc.sync.dma_start(
                                la[:, 0:ntl * 128],
                                reg_t[rk * (HID + 1):(rk + 1) * (HID + 1),
                                      ch * 128:(ch + ntl) * 128])
                        if reg == 0:
                            tab = tabs[rk // (M // 2)]
                            row0 = (rk % (M // 2)) * colr + ch * 128
                        else:
                            tab = tabs[2]
                            row0 = rk * colr + ch * 128
                        emit_kv(tab, row0, ntl, la)

                    def pa_items(lyr, part):
                        # part: 0 = tabA0 (ranks 0..M/2-1), 1 = tabA1, 2 = B
                        reg = 0 if part < 2 else 1
                        regt = (COLA if reg == 0 else COLB) // 128
                        rks = (range(part * (M // 2), (part + 1) * (M // 2))
                               if reg == 0 else range(M))
                        for rk in rks:
                            ch = 0
                            while ch < regt:
                                ntl = min(PA_CHUNK, regt - ch)
                                yield (lyr, reg, rk, ch, ntl)
                                ch += ntl

                    def emit_qs():
                        for ch in range(0, NBLK, PA_CHUNK):
                            ntl = min(PA_CHUNK, NBLK - ch)
                            if layer == 1:
                                la = pa.tile([D_IN, PA_CHUNK * 128], BF16,
                                             tag="la")
                                c0 = ch * 128
                                nc.sync.dma_start(
                                    la[:, 0:ntl * 128],
                                    inp["xoT"].ap()[:, c0:c0 + ntl * 128])
                            for t in range(ntl):
                                gt = ch + t
                                ps = pap.tile([128, KV], F32,
                                              tag="pskv", name="psqs")
                                if layer == 1:
                                    nc.tensor.matmul(
                                        ps[:, 0:F + HID],
                                        la[:, t * 128:(t + 1) * 128],
                                        W1_sb[:, KV:WC], start=True, stop=True)
                                else:
                                    nc.tensor.matmul(
                                        ps[:, 0:F + HID],
                                        hTown[:, gt * 128:(gt + 1) * 128],
                                        W_sb[:, KV:WC], start=True, stop=True)
                                nc.vector.tensor_copy(
                                    q_sb[:, gt * F:(gt + 1) * F],
                                    ps[:, 0:F])
                                nc.scalar.copy(
                                    s_sb[:, gt * HID:(gt + 1) * HID],
                                    ps[:, F:F + HID])
                            if layer == 1:
                                nc.vector.tensor_tensor(
                                    q_sb[:, ch * F:(ch + ntl) * F].rearrange(
                                        "p (t e) -> p t e", e=F),
                                    q_sb[:, ch * F:(ch + ntl) * F].rearrange(
                                        "p (t e) -> p t e", e=F),
                                    brep1_sb[:, KV:KV + F].rearrange(
                                        "p (t e) -> p t e", t=1).to_broadcast(
                                            [128, ntl, F]),
                                    op=OP.add)
                                nc.vector.tensor_tensor(
                                    s_sb[:, ch * HID:(ch + ntl) * HID
                                         ].rearrange("p (t e) -> p t e", e=HID),
                                    s_sb[:, ch * HID:(ch + ntl) * HID
                                         ].rearrange("p (t e) -> p t e", e=HID),
                                    brep1_sb[:, KV + F:WC].rearrange(
                                        "p (t e) -> p t e", t=1).to_broadcast(
                                            [128, ntl, HID]),
                                    op=OP.add)

                    emit_qs()
                    if layer == 1:
                        for it in pa_items(1, 0):
                            pa_item(*it)
                    # A0 of the NEXT layer is woven into this layer's g=2
                    # phase by the phase-B loop below; A1 and B of THIS layer
                    # are woven into g=0 / g=1.
                    weaveA1 = list(pa_items(layer, 1))
                    weaveB = list(pa_items(layer, 2))
                    weaveA0n = []   # next layer's A0, filled per chunk below

                # ---------- Phase B ----------
                if True:
                    psum_g = None
                    collA_issued = False
                    nxt = None
                    for cj, (t0, nt, g) in enumerate(chunks):
                        n = nt * TILE_E
                        tab = tabs[g]
                        # weave pending phase-A work between chunks
                        if g == 0:
                            for _ in range(2):
                                if weaveA1:
                                    pa_item(*weaveA1.pop(0))
                        elif g == 1:
                            if weaveA1:
                                while weaveA1:
                                    pa_item(*weaveA1.pop(0))
                            for _ in range(3):
                                if weaveB:
                                    pa_item(*weaveB.pop(0))
                        elif g == 2:
                            if weaveB:
                                while weaveB:
                                    pa_item(*weaveB.pop(0))
                            if collA_issued and layer < 3:
                                if nxt is None:
                                    nxt = list(pa_items(layer + 1, 0))
                                for _ in range(4):
                                    if nxt:
                                        W_next = {2: W2_sb, 3: W3_sb}[
                                            layer + 1]
                                        W_save = W_sb
                                        W_sb = W_next
                                        lyr_save = layer
                                        layer = layer + 1
                                        pa_item(*nxt.pop(0))
                                        layer = lyr_save
                                        W_sb = W_save
                        kvg = pb.tile([128, CHUNK_T, KV], BF16, tag="kvg")
                        rhs = pb.tile([128, CHUNK_T, RHSW], BF16, tag="rhs")
                        Sg = pb1.tile([128, CHUNK_T, 128], FP8, tag="Sg")
                        STg = pb1.tile([128, CHUNK_T, 128], FP8, tag="STg")
                        prod = pb1.tile([128, CHUNK_T * F], BF16, tag="prod")
                        alph = pb1.tile([128, CHUNK_T * H], F32, tag="alph")

                        if layer == 1 and cj < 3:
                            nc.vector.memset(
                                kvg[:].rearrange("p a b -> p (a b)"), 0.0)
                        nc.gpsimd.dma_gather(
                            out_ap=kvg[:, 0:nt, :], in_ap=tab[:],
                            idxs_ap=kvidx_sb[:, t0 * 8:t0 * 8 + nt * 8],
                            num_idxs=n, num_idxs_reg=n, elem_size=KV,
                            single_packet=False)
                        nc.sync.dma_start(
                            Sg[:, 0:nt, :].rearrange("p a b -> p (a b)"),
                            inp["S_in"].ap()[:, t0 * TILE_E:t0 * TILE_E + n])
                        nc.sync.dma_start(
                            STg[:, 0:nt, :].rearrange("p a b -> p (a b)"),
                            inp["ST_in"].ap()[:, t0 * TILE_E:t0 * TILE_E + n])

                        i = 0
                        while i < nt:
                            ng = min(QE_G, nt - i)
                            qeg = qep.tile([128, QE_G * F], F32, name="qeg",
                                           tag="qeg")
                            for j in range(ng):
                                b = tmeta[t0 + i + j][1]
                                nc.tensor.matmul(
                                    qeg[:, j * F:(j + 1) * F],
                                    STg[:, i + j, :],
                                    q_sb[:, b * F:(b + 1) * F],
                                    start=True, stop=True)
                            nc.vector.tensor_tensor(
                                out=prod[:, i * F:(i + ng) * F].rearrange(
                                    "p (t f) -> p t f", f=F),
                                in0=qeg[:, 0:ng * F].rearrange(
                                    "p (t f) -> p t f", f=F),
                                in1=kvg[:, i:i + ng, 0:F],
                                op=OP.mult)
                            i += ng
                        nc.vector.reduce_sum(
                            alph[:, 0:nt * H].rearrange("p (t h) -> p t h",
                                                        h=H),
                            prod[:, 0:nt * F].rearrange(
                                "p (t h c) -> p t h c", h=H, c=C),
                            axis=mybir.AxisListType.X)
                        nc.scalar.activation(
                            rhs[:, 0:nt, 0:H],
                            alph[:, 0:nt * H].rearrange("p (t h) -> p t h",
                                                        h=H),
                            AF.Exp)
                        nc.vector.tensor_tensor(
                            out=rhs[:, 0:nt, H:RHSW].rearrange(
                                "p t (h c) -> p t h c", c=C),
                            in0=kvg[:, 0:nt, F:KV].rearrange(
                                "p t (h c) -> p t h c", c=C),
                            in1=rhs[:, 0:nt, 0:H].to_broadcast(
                                [128, nt, H, C]),
                            op=OP.mult)

                        for i in range(nt):
                            gg, b, st, sp = tmeta[t0 + i]
                            if st:
                                psum_g = pbp.tile([128, RHSW], F32,
                                                  name="pblk", tag="pblk")
                            nc.tensor.matmul(
                                psum_g[:], Sg[:, i, :], rhs[:, i, :],
                                start=st, stop=sp)
                            if not sp:
                                continue
                            pa_sl = partA[:, b * RHSW:(b + 1) * RHSW]
                            if gg == 0:
                                nc.vector.tensor_copy(pa_sl, psum_g[:])
                                continue
                            if gg < NG - 1:
                                nc.vector.tensor_tensor(pa_sl, psum_g[:],
                                                        pa_sl, op=OP.add)
                                continue
                            # ---- epilogue for block b ----
                            tot = ep.tile([128, RHSW], F32, tag="tot")
                            nc.vector.tensor_tensor(tot[:], psum_g[:], pa_sl,
                                                    op=OP.add)
                            rec = ep.tile([128, H], F32, tag="rec")
                            nc.vector.scalar_tensor_tensor(
                                out=rec[:], in0=tot[:, 0:H], scalar=float(H),
                                in1=eps2[:], op0=OP.mult, op1=OP.add)
                            nc.vector.reciprocal(rec[:], rec[:])
                            m0 = ep.tile([128, C], F32, tag="m0")
                            nc.vector.scalar_tensor_tensor(
                                out=m0[:], in0=tot[:, H:H + C],
                                scalar=rec[:, 0:1],
                                in1=s_sb[:, b * HID:(b + 1) * HID],
                                op0=OP.mult, op1=OP.add)
                            hp2 = ep.tile([128, HID], F32, tag="hp2")
                            nc.vector.scalar_tensor_tensor(
                                out=hp2[:], in0=tot[:, H + C:H + 2 * C],
                                scalar=rec[:, 1:2], in1=m0[:],
                                op0=OP.mult, op1=OP.add)
                            hblk = ep.tile([128, HID], F32, tag="hblk")
                            nc.scalar.activation(hblk[:], hp2[:], AF.Relu)
                            if layer < 3:
                                pst = epp.tile([HID, 128], F32)
                                nc.tensor.transpose(pst[:], hblk[:], ident[:])
                                nc.vector.tensor_copy(
                                    hTown[0:HID, b * 128:(b + 1) * 128],
                                    pst[:])
                                if b == SPLIT_B - 1:
                                    nc.sync.dma_start(coll_inA[:, :],
                                                      hTown[:, 0:COLA])
                                    nc.gpsimd.collective_compute(
                                        "AllGather", OP.bypass,
                                        ins=[coll_inA.opt()],
                                        outs=[coll_outA.opt()],
                                        replica_groups=[list(range(M))])
                                    collA_issued = True
                            else:
                                nc.sync.dma_start(
                                    h_out.ap()[b * 128:(b + 1) * 128, :],
                                    hblk[:])

                if layer < 3:
                    nc.sync.dma_start(coll_inB[:, :], hTown[:, COLA:])
                    nc.gpsimd.collective_compute(
                        "AllGather", OP.bypass,
                        ins=[coll_inB.opt()], outs=[coll_outB.opt()],
                        replica_groups=[list(range(M))])
                    if nxt is None:
                        nxt = list(pa_items(layer + 1, 0))
                    if nxt:
                        W_next = {2: W2_sb, 3: W3_sb}[layer + 1]
                        W_save = W_sb
                        W_sb = W_next
                        lyr_save = layer
                        layer = layer + 1
                        while nxt:
                            pa_item(*nxt.pop(0))
                        layer = lyr_save
                        W_sb = W_save
    nc.compile()
    return nc


# ---------------- public entry ----------------
_CACHE = {}


def _weights_from_inputs(inputs, d):
    # packed column order: k | v | q | s ; q pre-scaled by 1/sqrt(C)
    sc = 1.0 / np.sqrt(d["C"])
    wt = {}
    for L in (1, 2, 3):
        Ws, bs = [], []
        for nm in ("k", "v", "q", "s"):
            W = np.asarray(inputs[f"W{L}{nm}"], np.float32)
            b = np.asarray(inputs[f"b{L}{nm}"], np.float32)
            if nm == "q":
                W = W * sc
                b = b * sc
            Ws.append(W)
            bs.append(b)
        wt[L] = (np.concatenate(Ws, axis=1), np.concatenate(bs))
    return wt


def _install_ntff_shim():
    import types
    if "antenv.axon_hooks" in sys.modules:
        return
    try:
        from trn_agent_boot.trn_boot import _ntff_profile_via_ctypes
        hook = _ntff_profile_via_ctypes("/opt/axon/libaxon_pjrt.so")
    except Exception:
        hook = None
    mod = types.ModuleType("antenv.axon_hooks")
    mod.get_axon_ntff_profile_hook = lambda: hook
    mod.set_axon_ntff_profile_hook = lambda h: None
    sys.modules["antenv.axon_hooks"] = mod
    try:
        import antenv
        antenv.axon_hooks = mod
    except Exception:
        pass


def run(inputs, cfg=SPEC, trace=False):
    d = _derive(cfg)
    wt = _weights_from_inputs(inputs, d)
    in_maps, meta = _prep(inputs["x"], inputs["edge_index"], wt, d)
    for m in range(d["M"]):
        in_maps[m]["xoT"] = np.ascontiguousarray(
            in_maps[m]["xT"][:, m * d["NPC"]:(m + 1) * d["NPC"]])
    key = (tuple(sorted(cfg.items())), meta["TT"],
           tuple(meta["T"].flatten().tolist()))
    if key not in _CACHE:
        _CACHE[key] = build_module(d, meta)
    nc = _CACHE[key]
    if trace:
        _install_ntff_shim()
    res = bass_utils.run_bass_kernel_spmd(
        nc, in_maps, core_ids=list(range(d["M"])), trace=trace)
    pid = meta["pid"]
    N, NPC = d["N"], d["NPC"]
    full = np.empty((N, d["HID"]), np.float32)
    for m in range(d["M"]):
        sel = np.arange(m * d["NPC_REAL"], (m + 1) * d["NPC_REAL"])
        full[sel] = res.results[m]["h_out"][pid[sel] - m * NPC]
    return full, res


def kernel(**inputs) -> np.ndarray:
    trace = bool(os.environ.get("KERNEL_TRACE"))
    full, res = run(inputs, SPEC, trace=trace)
    if trace and res.exec_time_ns is not None:
        print(f"HW exec time: {res.exec_time_ns} ns")
    return full
